# revision 1
# baseline (speedup 1.0000x reference)
"""Trainium2 Bass kernel for nn_NewActivationGNN (GNN message passing).

Redesign vs the dma_gather baseline: all per-edge gathers are gpsimd
ap_gather reads from an SBUF-resident, feature-major (transposed) copy of
the hidden-state table — no SWDGE descriptors at all. Gathered columns are
transposed back to slot-major by PE matmuls that fuse the layer weight
(rhs = W_l instead of identity), then selection-matrix matmuls accumulate
destination packs in PSUM; an SBUF f32 accumulator integrates the 8 source
passes. The whole pipeline (projection, activation, residual, output) runs
in feature-major orientation.

Sharding: nodes split across 8 cores by destination (graph parallel);
per-layer AllGather exchanges the fp16 table slices; weights replicated.
SPMD: one program for all cores; per-device variation (indices, S
matrices, features) is data. Chunk counts / S spans are made
device-uniform by padding to the cross-device maximum.

Gather mechanics: the fp16 table is bitcast to f32 so each 4-byte gather
moves a pair of adjacent nodes (2m, 2m+1); edge slots are grouped by source
parity so a stride-2 fp16 view of the gathered buffer exposes exactly the
wanted node per slot.
"""

import sys

for _p in ("/opt/trn_rl_repo", "/root/.axon_site/_ro/trn_rl_repo"):
    if _p not in sys.path:
        sys.path.insert(0, _p)

from dataclasses import dataclass

import numpy as np

import concourse.bass as bass  # noqa: F401
import concourse.tile as tile
from concourse import bacc, mybir
from concourse.masks import make_identity

P = 128


@dataclass
class Cfg:
    N: int = 50000
    E: int = 800000
    NFEAT: int = 500
    NHID: int = 128
    NCLASS: int = 40
    NLAYERS: int = 4
    GAMMA: float = 0.3
    X1: float = 0.1
    X2: float = 0.9
    C_ACT: float = -1.0
    n_cores: int = 8
    PACK: int = 512

    @property
    def R(self):
        return self.N // self.n_cores          # 6250 dest rows per core

    @property
    def NPACK(self):
        return (self.R + self.PACK - 1) // self.PACK   # 13

    @property
    def NPASS(self):
        return self.n_cores                    # source passes = 8

    @property
    def NFP(self):
        return ((self.NFEAT + 1 + P - 1) // P) * P     # 512


class Sched:
    """Device-uniform schedule.

    cells: (pass, pack, parity) -> chunk list; each chunk is 128 slots of
    the idx stream. Per chunk: (col_off, width, s_off) for the S matmul
    (uniform across devices), plus idx-stream offsets per cell.
    """

    def __init__(self, cfg, counts, spans):
        # counts: [ncores, NPASS, NPACK, 2]; spans computed by caller
        self.B = counts.max(axis=0)                      # slots budget/cell
        self.nch = (self.B + P - 1) // P                 # chunks per cell
        self.cell_off = {}    # (p,k,par) -> idx-stream slot offset
        self.chunk_meta = []  # flat list per (p,k,par,ci): dict
        self.pass_off = []    # slot offset where each pass begins
        self.pass_slots = []
        off = 0
        for p in range(cfg.NPASS):
            self.pass_off.append(off)
            for k in range(cfg.NPACK):
                for par in range(2):
                    self.cell_off[(p, k, par)] = off
                    off += int(self.nch[p, k, par]) * P
            self.pass_slots.append(off - self.pass_off[-1])
        self.idx_total = off
        # S layout: s_off cumulative in (p, k, par, ci) order
        s_off = 0
        self.s_cell = {}
        for p in range(cfg.NPASS):
            for k in range(cfg.NPACK):
                for par in range(2):
                    metas = []
                    for ci in range(int(self.nch[p, k, par])):
                        co, w = spans[(p, k, par, ci)]
                        metas.append((co, w, s_off))
                        s_off += w
                    self.s_cell[(p, k, par)] = metas
        self.s_total = s_off


def preprocess(cfg: Cfg, features, edge_row, edge_col, W_in, Ws, c, W_out):
    N, R, ncores = cfg.N, cfg.R, cfg.n_cores
    NPASS, NPACK, PACK = cfg.NPASS, cfg.NPACK, cfg.PACK
    f32 = np.float32

    deg = np.bincount(edge_row, minlength=N)
    deg_inv = (1.0 / np.maximum(deg, 1)).astype(f32)
    owner = edge_row // R

    # Node permutation: within each device, order nodes by descending global
    # degree so cumulative degree profiles align across devices (smaller S
    # spans, less cell padding). pos[] maps orig node id -> its table
    # position within the owning device's slice.
    dest_of = np.empty((ncores, R), np.int64)   # sorted pos -> orig local
    pos = np.empty(N, np.int64)
    for d in range(ncores):
        order = np.argsort(-deg[d * R:(d + 1) * R], kind="stable")
        dest_of[d] = order
        local_of = np.empty(R, np.int64)
        local_of[order] = np.arange(R)
        pos[d * R:(d + 1) * R] = local_of

    dev = []
    counts = np.zeros((ncores, NPASS, NPACK, 2), np.int64)
    for d in range(ncores):
        m = owner == d
        dl = pos[edge_row[m]]
        src = edge_col[m].astype(np.int64)
        p = src // R
        lm = pos[src]              # position within source device's slice
        par = lm % 2
        iv = lm // 2
        k = dl // PACK
        so = np.lexsort((dl, par, k, p))
        dl, p, par, iv, k = dl[so], p[so], par[so], iv[so], k[so]
        cell_id = (p * NPACK + k) * 2 + par
        cnt = np.bincount(cell_id, minlength=NPASS * NPACK * 2)
        counts[d] = cnt.reshape(NPASS, NPACK, 2)
        dev.append((dl, iv, cell_id))

    B = counts.max(axis=0)
    nch = (B + P - 1) // P

    # chunk spans (cross-device)
    spans = {}
    # per-device cell start offsets within its own sorted arrays
    dev_cell_start = []
    for d in range(ncores):
        cnt = counts[d].reshape(-1)
        dev_cell_start.append(np.concatenate([[0], np.cumsum(cnt)]))
    for p in range(NPASS):
        for k in range(NPACK):
            for par in range(2):
                cid = (p * NPACK + k) * 2 + par
                for ci in range(int(nch[p, k, par])):
                    fd, ld = [], []
                    for d in range(ncores):
                        dl = dev[d][0]
                        b = dev_cell_start[d][cid]
                        n_d = counts[d, p, k, par]
                        s0 = ci * P
                        if s0 < n_d:
                            s1 = min(s0 + P, n_d)
                            fd.append(int(dl[b + s0]))
                            ld.append(int(dl[b + s1 - 1]))
                    co = min(fd) - k * PACK
                    w = max(ld) - min(fd) + 1
                    spans[(p, k, par, ci)] = (co, w)

    sch = Sched(cfg, counts, spans)

    # shared weights
    NFP = cfg.NFP
    W_aug = np.zeros((NFP, cfg.NHID), f32)
    W_aug[:cfg.NFEAT] = (1.0 - cfg.GAMMA) * W_in
    W_aug[cfg.NFEAT] = cfg.GAMMA * np.maximum(c, 0.0)
    nk = NFP // P
    W_dram = np.empty((P, nk * P), np.float16)
    for kk in range(nk):
        W_dram[:, kk * P:(kk + 1) * P] = W_aug[kk * P:(kk + 1) * P]
    Ws_dram = np.empty((P, cfg.NLAYERS * P), np.float16)
    for l in range(cfg.NLAYERS):
        Ws_dram[:, l * P:(l + 1) * P] = Ws[l]
    Wout_dram = np.ascontiguousarray(W_out).astype(np.float16)

    in_maps = []
    for d in range(ncores):
        dl, iv, cell_id = dev[d]
        cstart = dev_cell_start[d]
        idx_vals = np.zeros(sch.idx_total, np.int16)
        s_data = np.zeros((P, sch.s_total), np.float16)
        for p in range(NPASS):
            for k in range(NPACK):
                for par in range(2):
                    cid = (p * NPACK + k) * 2 + par
                    n_d = int(counts[d, p, k, par])
                    if n_d == 0:
                        continue
                    b = cstart[cid]
                    o = sch.cell_off[(p, k, par)]
                    idx_vals[o:o + n_d] = iv[b:b + n_d].astype(np.int16)
                    # S fill
                    metas = sch.s_cell[(p, k, par)]
                    sl = np.arange(n_d)
                    ci_arr = sl // P
                    row = sl % P
                    co = np.array([m[0] for m in metas], np.int64)[ci_arr]
                    soff = np.array([m[2] for m in metas], np.int64)[ci_arr]
                    col = soff + (dl[b:b + n_d] - k * PACK - co)
                    s_data[row, col] = deg_inv[
                        d * R + dest_of[d][dl[b:b + n_d]]]
        idx_t = np.tile(idx_vals.reshape(-1, 16).T, (8, 1))

        gids = d * R + dest_of[d]
        featT = np.zeros((NFP, R), np.float16)
        featT[:cfg.NFEAT] = features[gids].T
        featT[cfg.NFEAT] = 1.0

        in_maps.append(dict(
            featT=featT, idx_all=np.ascontiguousarray(idx_t), s_all=s_data,
            w_proj=W_dram, w_hid=Ws_dram, w_out=Wout_dram,
        ))
    return in_maps, sch, dest_of


def build_program(cfg: Cfg, sch: Sched, rep: int = 1):
    nc = bacc.Bacc("TRN2", target_bir_lowering=False, debug=False,
                   num_devices=cfg.n_cores, num_swdge_queues=1)
    DT = mybir.dt.float16
    f32 = mybir.dt.float32
    R, NPACK, NPASS, NFP = cfg.R, cfg.NPACK, cfg.NPASS, cfg.NFP
    AFT = mybir.ActivationFunctionType
    ALU = mybir.AluOpType
    AX = mybir.AxisListType
    RED = bass.bass_isa.ReduceOp
    rg = [list(range(cfg.n_cores))]
    nk = NFP // P
    NCHK = (R + cfg.PACK - 1) // cfg.PACK  # 512-col chunks over R

    def cw(ch):  # chunk width
        return min(cfg.PACK, R - ch * cfg.PACK)

    featT = nc.dram_tensor("featT", [NFP, R], DT, kind="ExternalInput").ap()
    idx_all = nc.dram_tensor("idx_all", [P, sch.idx_total // 16],
                             mybir.dt.int16, kind="ExternalInput").ap()
    s_all = nc.dram_tensor("s_all", [P, sch.s_total], DT,
                           kind="ExternalInput").ap()
    w_proj = nc.dram_tensor("w_proj", [P, nk * P], DT,
                            kind="ExternalInput").ap()
    w_hid = nc.dram_tensor("w_hid", [P, cfg.NLAYERS * P], DT,
                           kind="ExternalInput").ap()
    w_out = nc.dram_tensor("w_out", [P, cfg.NCLASS], DT,
                           kind="ExternalInput").ap()
    out = nc.dram_tensor("out", [R, cfg.NCLASS], f32,
                         kind="ExternalOutput").ap()

    INV08 = float(np.float32(1.0 / (np.float64(cfg.X2) - cfg.X1 + 1e-8)))
    B_RELU = float(np.float32(-cfg.X1 * INV08))
    E1 = float(1.0 + np.exp(-cfg.C_ACT))
    NIT = cfg.NLAYERS * rep

    # gather call split: per pass, group cells into calls of <= GCAP slots
    GCAP = 3328
    calls = []   # per pass: list of (slot_off_rel, n_slots, [cell keys])
    for p in range(NPASS):
        groups = []
        cur, cur_n = [], 0
        for k in range(NPACK):
            for par in range(2):
                n = int(sch.nch[p, k, par]) * P
                if n == 0:
                    cur.append((k, par))
                    continue
                if cur_n + n > GCAP and cur_n > 0:
                    groups.append((cur, cur_n))
                    cur, cur_n = [], 0
                cur.append((k, par))
                cur_n += n
        if cur:
            groups.append((cur, cur_n))
        calls.append(groups)
    GMAX = max(n for gs in calls for (_, n) in gs)

    with tile.TileContext(nc) as tc:
        with tc.tile_pool(name="persist", bufs=1) as persist, \
             tc.tile_pool(name="dram", bufs=1, space="DRAM") as dram:
            idx_sb = persist.tile([P, sch.idx_total // 16], mybir.dt.int16)
            nc.sync.dma_start(idx_sb[:], idx_all[:])
            x0T = persist.tile([P, R], DT)      # normalized x0, beta-scaled
            yT = persist.tile([P, R], f32)      # spmm accumulator / proj scratch
            wh_sb = persist.tile([P, cfg.NLAYERS * P], DT)
            nc.sync.dma_start(wh_sb[:], w_hid[:])
            wo_sb = persist.tile([P, cfg.NCLASS], DT)
            nc.sync.dma_start(wo_sb[:], w_out[:])
            wp_sb = persist.tile([P, nk * P], DT)
            nc.sync.dma_start(wp_sb[:], w_proj[:])
            zero1 = persist.tile([1, P], DT)
            nc.vector.memset(zero1[:], 0.0)
            zero512 = persist.tile([1, 512], DT)
            nc.vector.memset(zero512[:], 0.0)
            ones1 = persist.tile([1, P], f32)
            nc.vector.memset(ones1[:], 1.0)
            b_relu = persist.tile([P, 1], f32)
            nc.vector.memset(b_relu[:], B_RELU)
            idn = persist.tile([P, P], DT)
            make_identity(nc, idn[:])
            mm_sb = persist.tile([P, 2], f32)
            mm_ar = persist.tile([P, 2], f32)
            mm_back = persist.tile([1, 2], f32)
            sfac = persist.tile([P, 1], f32)
            bfac = persist.tile([P, 1], f32)

            bounce = [dram.tile([P, R], DT, name=f"bounce{i}")
                      for i in range(NIT)]
            x_full = [dram.tile([P * NPASS, R], DT, addr_space="Shared",
                                name=f"x_full{i}") for i in range(NIT)]
            mm_in = dram.tile([1, 2], f32)
            mm_out = dram.tile([1, 2], f32, addr_space="Shared")

            # ================= projection =================
            with tc.tile_pool(name="strips", bufs=1) as strip_pool, \
                 tc.tile_pool(name="pwork", bufs=2) as pwork, \
                 tc.tile_pool(name="pps", bufs=2, space="PSUM") as pps_pool:
                strips = []
                for k in range(nk):
                    st = strip_pool.tile([P, R], DT, name=f"strip{k}",
                                         tag=f"strip{k}")
                    nc.sync.dma_start(st[:], featT[k * P:(k + 1) * P, :])
                    strips.append(st)
                rmax = pwork.tile([P, 1], f32, name="rmax", tag="rmax")
                rmin = pwork.tile([P, 1], f32, name="rmin", tag="rmin")
                for ch in range(NCHK):
                    w = cw(ch)
                    sl = slice(ch * cfg.PACK, ch * cfg.PACK + w)
                    ps = pps_pool.tile([P, cfg.PACK], f32, name=f"h{ch}",
                                       tag="hps")
                    for k in range(nk):
                        nc.tensor.matmul(ps[:, :w],
                                         lhsT=wp_sb[:, k * P:(k + 1) * P],
                                         rhs=strips[k][:, sl],
                                         start=(k == 0), stop=(k == nk - 1))
                    nc.vector.tensor_copy(yT[:, sl], ps[:, :w])
                    qmax = pwork.tile([P, 1], f32, name="qmax", tag="qmax")
                    qmin = pwork.tile([P, 1], f32, name="qmin", tag="qmin")
                    nc.vector.tensor_reduce(qmax[:], ps[:, :w], axis=AX.X,
                                            op=ALU.max)
                    nc.vector.tensor_reduce(qmin[:], ps[:, :w], axis=AX.X,
                                            op=ALU.min)
                    if ch == 0:
                        nc.vector.tensor_copy(rmax[:], qmax[:])
                        nc.vector.tensor_copy(rmin[:], qmin[:])
                    else:
                        nc.vector.tensor_tensor(rmax[:], rmax[:], qmax[:],
                                                op=ALU.max)
                        nc.vector.tensor_tensor(rmin[:], rmin[:], qmin[:],
                                                op=ALU.min)
                nc.vector.tensor_copy(mm_sb[:, 0:1], rmax[:])
                nc.vector.tensor_scalar(mm_sb[:, 1:2], rmin[:], -1.0, None,
                                        ALU.mult)
                nc.gpsimd.partition_all_reduce(mm_ar[:], mm_sb[:],
                                               channels=P, reduce_op=RED.max)
                nc.sync.dma_start(mm_in[:], mm_ar[0:1, :])
                nc.gpsimd.collective_compute(
                    "AllReduce", ALU.max, ins=[mm_in.opt()],
                    outs=[mm_out.opt()], replica_groups=rg)
                nc.sync.dma_start(mm_back[:], mm_out[:])
                bc_ps = pps_pool.tile([P, 2], f32, name="bc_ps", tag="hps")
                nc.tensor.matmul(bc_ps[:], lhsT=ones1[:], rhs=mm_back[:],
                                 start=True, stop=True)
                bcast = pwork.tile([P, 2], f32, name="bcast", tag="qmin")
                nc.vector.tensor_copy(bcast[:], bc_ps[:])
                sden = pwork.tile([P, 1], f32, name="sden", tag="qmax")
                nc.vector.tensor_tensor(sden[:], bcast[:, 0:1], bcast[:, 1:2],
                                        op=ALU.add)
                nc.vector.tensor_scalar(sden[:], sden[:], 1e-8, None, ALU.add)
                nc.vector.reciprocal(sfac[:], sden[:])
                nc.vector.tensor_tensor(bfac[:], bcast[:, 1:2], sfac[:],
                                        op=ALU.mult)
                nc.vector.tensor_scalar(x0T[:], yT[:], sfac[:], bfac[:],
                                        ALU.mult, ALU.add)
                nc.sync.dma_start(bounce[0][:], x0T[:])
            nc.gpsimd.collective_compute(
                "AllGather", ALU.bypass, ins=[bounce[0].opt()],
                outs=[x_full[0].opt()], replica_groups=rg)

            # ================= conv layers =================
            with tc.tile_pool(name="tpool", bufs=3) as tpool, \
                 tc.tile_pool(name="gpool", bufs=3) as gpool, \
                 tc.tile_pool(name="spool", bufs=3) as spool, \
                 tc.tile_pool(name="zwork", bufs=4) as zwork, \
                 tc.tile_pool(name="lwork", bufs=1) as lwork, \
                 tc.tile_pool(name="zps", bufs=4, space="PSUM") as zps_pool, \
                 tc.tile_pool(name="pps2", bufs=2, space="PSUM") as pps2_pool:
                beta_prev = 1.0
                for l in range(NIT):
                    li = l % cfg.NLAYERS
                    last = l == NIT - 1
                    beta = min(0.5, (li + 1) / cfg.NLAYERS * 0.5)
                    c1 = float((1.0 - beta) * E1)
                    # progressive in-place rescale: x0T *= beta/beta_prev
                    nc.vector.tensor_scalar(x0T[:], x0T[:],
                                            float(beta / beta_prev), None,
                                            ALU.mult)
                    beta_prev = beta
                    for p in range(NPASS):
                        tbl = tpool.tile([P, R], DT, name=f"t{l}_{p}",
                                         tag="tbl")
                        nc.sync.dma_start(tbl[:],
                                          x_full[l][p * P:(p + 1) * P, :])
                        tbl3 = tbl[:].bitcast(f32).rearrange(
                            "p (n one) -> p n one", one=1)
                        # S slices for this pass: pack groups of <=SCAP cols
                        SCAP = 4096
                        s_parts = []   # (k_lo, k_hi, s_lo, tile)
                        k0 = 0
                        while k0 < NPACK:
                            k1 = k0 + 1
                            def _ext(ka, kb):
                                smet = [sch.s_cell[(p, k, par)]
                                        for k in range(ka, kb)
                                        for par in range(2)]
                                lo = min((m[0][2] for m in smet if m),
                                         default=0)
                                hi = max((m[-1][2] + m[-1][1]
                                          for m in smet if m), default=0)
                                return lo, hi
                            while k1 < NPACK:
                                lo, hi = _ext(k0, k1 + 1)
                                if hi - lo > SCAP:
                                    break
                                k1 += 1
                            lo, hi = _ext(k0, k1)
                            if hi > lo:
                                st = spool.tile([P, hi - lo], DT,
                                                name=f"s{l}_{p}_{k0}", tag="s")
                                nc.sync.dma_start(st[:], s_all[:, lo:hi])
                            else:
                                st = None
                            s_parts.append((k0, k1, lo, st))
                            k0 = k1
                        # gather calls
                        gts = {}   # (k, par) -> (tile, slot offset in tile)
                        for (cells, n_slots) in calls[p]:
                            gt = gpool.tile([P, GMAX], f32,
                                            name=f"g{l}_{p}_{cells[0]}",
                                            tag="g")
                            base = sch.cell_off[(p, cells[0][0], cells[0][1])]
                            for (k, par) in cells:
                                gts[(k, par)] = (
                                    gt, sch.cell_off[(p, k, par)] - base)
                            if n_slots > 0:
                                nc.gpsimd.ap_gather(
                                    out_ap=gt[:, :n_slots].rearrange(
                                        "p (n one) -> p n one", one=1),
                                    in_ap=tbl3,
                                    idxs_ap=idx_sb[:, base // 16:
                                                   (base + n_slots) // 16],
                                    channels=P, num_elems=R // 2, d=1,
                                    num_idxs=n_slots)
                        for kp in range(0, NPACK, 2):
                            pair = [k for k in (kp, kp + 1) if k < NPACK]
                            wpair = sum(cw(k) for k in pair)
                            n_ch = sum(int(sch.nch[p, k, par])
                                       for k in pair for par in range(2))
                            pps = pps2_pool.tile([P, 2 * cfg.PACK], f32,
                                                 name=f"pp{l}_{p}_{kp}",
                                                 tag="pps")
                            # zero on DVE (PE is the busier engine); chunk
                            # matmuls then accumulate with start=False
                            nc.vector.memset(pps[:, :wpair], 0.0)
                            ci_flat = 0
                            for k in pair:
                                wk = cw(k)
                                kb = (k - kp) * cfg.PACK
                                s_lo, s_sb = next(
                                    (lo, st) for (k0, k1, lo, st) in s_parts
                                    if k0 <= k < k1)
                                for par in range(2):
                                    metas = sch.s_cell[(p, k, par)]
                                    if not metas:
                                        continue
                                    gt, goff = gts[(k, par)]
                                    gt16 = gt[:].bitcast(DT)
                                    nzz = len(metas)
                                    # z-transpose in batches of 4 chunks
                                    zbs = []
                                    for cb in range(0, nzz, 4):
                                        nb = min(4, nzz - cb)
                                        zp = zps_pool.tile(
                                            [P, cfg.PACK], f32,
                                            name=f"z{p}_{k}_{par}_{cb}",
                                            tag="zp")
                                        for j in range(nb):
                                            s0 = goff + (cb + j) * P
                                            lv = gt16[:, 2 * s0:2 * (s0 + P)] \
                                                .rearrange(
                                                    "p (n two) -> p n two",
                                                    two=2)[:, :, par]
                                            nc.tensor.matmul(
                                                zp[:, j * P:(j + 1) * P],
                                                lhsT=lv,
                                                rhs=wh_sb[:, li * P:
                                                          (li + 1) * P],
                                                start=True, stop=True)
                                        zb = zwork.tile(
                                            [P, cfg.PACK], DT,
                                            name=f"zb{p}_{k}_{par}_{cb}",
                                            tag="zb")
                                        if (cb // 4) % 2 == 0:
                                            nc.scalar.activation(
                                                zb[:, :nb * P], zp[:, :nb * P],
                                                AFT.Copy)
                                        else:
                                            nc.vector.tensor_copy(
                                                zb[:, :nb * P], zp[:, :nb * P])
                                        zbs.append(zb)
                                    for ci, (co, wdt, soff) in enumerate(metas):
                                        zb = zbs[ci // 4]
                                        jj = ci % 4
                                        wdt2 = min(wdt, wk - co)
                                        nc.tensor.matmul(
                                            pps[:, kb + co:kb + co + wdt2],
                                            lhsT=zb[:, jj * P:(jj + 1) * P],
                                            rhs=s_sb[:, soff - s_lo:
                                                     soff - s_lo + wdt2],
                                            start=False,
                                            stop=(ci_flat == n_ch - 1),
                                            skip_group_check=True)
                                        ci_flat += 1
                            ksl = slice(kp * cfg.PACK, kp * cfg.PACK + wpair)
                            if p == 0:
                                nc.vector.tensor_copy(yT[:, ksl],
                                                      pps[:, :wpair])
                            else:
                                nc.vector.tensor_tensor(yT[:, ksl], yT[:, ksl],
                                                        pps[:, :wpair],
                                                        op=ALU.add)
                    # activation chain (feature-major, full width, fp16)
                    a1 = lwork.tile([P, R], DT, name=f"a1_{l}", tag="a1")
                    nc.scalar.activation(a1[:], yT[:], AFT.Relu,
                                         bias=b_relu[:], scale=INV08)
                    nc.vector.tensor_scalar(a1[:], a1[:], 1.0, c1,
                                            ALU.min, ALU.mult)
                    # sigmoid output reuses yT's storage as fp16 scratch
                    s16 = yT[:].bitcast(DT)[:, :R]
                    nc.scalar.activation(s16, a1[:], AFT.Sigmoid,
                                         scale=float(-1.0 / c1))
                    nc.vector.tensor_tensor(a1[:], a1[:], s16, op=ALU.mult)
                    xn16 = lwork.tile([P, R], DT, name=f"xn{l}", tag="xn")
                    nc.vector.tensor_tensor(xn16[:], a1[:], x0T[:],
                                            op=ALU.add)
                    if not last:
                        nc.sync.dma_start(bounce[l + 1][:], xn16[:])
                        nc.gpsimd.collective_compute(
                            "AllGather", ALU.bypass, ins=[bounce[l + 1].opt()],
                            outs=[x_full[l + 1].opt()], replica_groups=rg)
                    else:
                        # ---- output stage ----
                        with tc.tile_pool(name="ow", bufs=2) as ow:
                            lgT = persist.tile([P, R], DT)
                            for ch in range(NCHK):
                                w = cw(ch)
                                sl = slice(ch * cfg.PACK, ch * cfg.PACK + w)
                                lg = zps_pool.tile([P, cfg.PACK], f32,
                                                   name=f"lg{ch}", tag="zp")
                                nc.tensor.matmul(lg[:cfg.NCLASS, :w],
                                                 lhsT=wo_sb[:],
                                                 rhs=xn16[:, sl],
                                                 start=True, stop=True)
                                nc.scalar.activation(lgT[:cfg.NCLASS, sl],
                                                     lg[:cfg.NCLASS, :w],
                                                     AFT.Copy)
                            for q in range(R // P + (1 if R % P else 0)):
                                r0 = q * P
                                w = min(P, R - r0)
                                tp = zps_pool.tile([P, cfg.NCLASS], f32,
                                                   name=f"tp{q}", tag="zp")
                                nc.tensor.matmul(
                                    tp[:w, :],
                                    lhsT=lgT[:cfg.NCLASS, r0:r0 + w],
                                    rhs=idn[:cfg.NCLASS, :cfg.NCLASS],
                                    start=True, stop=True)
                                mx = ow.tile([P, 1], f32, name="mx", tag="mx")
                                nc.vector.tensor_reduce(mx[:w], tp[:w, :],
                                                        axis=AX.X, op=ALU.max)
                                nmx = ow.tile([P, 1], f32, name="nmx",
                                              tag="nmx")
                                nc.vector.tensor_scalar(nmx[:w], mx[:w], -1.0,
                                                        None, ALU.mult)
                                ex = ow.tile([P, cfg.NCLASS], f32, name="ex",
                                             tag="ex")
                                sume = ow.tile([P, 1], f32, name="sume",
                                               tag="sume")
                                nc.scalar.activation(ex[:w], tp[:w, :],
                                                     AFT.Exp, bias=nmx[:w],
                                                     accum_out=sume[:w])
                                lse = ow.tile([P, 1], f32, name="lse",
                                              tag="lse")
                                nc.scalar.activation(lse[:w], sume[:w],
                                                     AFT.Ln)
                                nc.vector.tensor_tensor(lse[:w], lse[:w],
                                                        nmx[:w],
                                                        op=ALU.subtract)
                                res = ow.tile([P, cfg.NCLASS], f32,
                                              name="res", tag="ex")
                                nc.vector.tensor_scalar(res[:w], tp[:w, :],
                                                        lse[:w], None,
                                                        ALU.subtract)
                                nc.sync.dma_start(out[r0:r0 + w, :], res[:w])
    nc.compile()
    return nc


def kernel(**inputs) -> np.ndarray:
    cfg = Cfg()
    features = np.asarray(inputs["features"], np.float32)
    edge_row = np.asarray(inputs["edge_row"], np.int64)
    edge_col = np.asarray(inputs["edge_col"], np.int64)
    W_in = np.asarray(inputs["W_in"], np.float32)
    Ws = np.asarray(inputs["Ws"], np.float32)
    c = np.asarray(inputs["c"], np.float32)
    W_out = np.asarray(inputs["W_out"], np.float32)

    in_maps, sch, dest_of = preprocess(cfg, features, edge_row, edge_col,
                                       W_in, Ws, c, W_out)
    nc = build_program(cfg, sch)

    import os
    from concourse import bass_utils
    res = bass_utils.run_bass_kernel_spmd(
        nc, in_maps, core_ids=list(range(cfg.n_cores)),
        trace=bool(os.environ.get("GNN_TRACE")))
    kernel.last_result = res
    out = np.empty((cfg.N, cfg.NCLASS), np.float32)
    for d in range(cfg.n_cores):
        out[d * cfg.R + dest_of[d]] = res.results[d]["out"]
    return out



# revision 13
# speedup vs baseline: 6.2428x; 6.2428x over previous
"""Trainium2 Bass kernel for nn_NewActivationGNN (GNN message passing).

Architecture (v2, swdge gather):
  y_l = A_norm @ (x @ W_l) is computed as (A_norm @ x) @ W_l — aggregate
  raw hidden rows first, transform after. Per-edge source rows are pulled
  straight from the DRAM-resident AllGathered node table with SWDGE
  dma_gather (one 256B descriptor per edge, slot-major destination), so
  the gather costs ~23ns/edge across 16 SDMA engines instead of ~26ns/edge
  serialized on gpsimd Q7 cores.

  Slot-major pipeline: gathered chunks [128 edge-slots, 128 hid] are lhsT
  for selection-matrix matmuls (rhs = S fp16 [slot, dest-span] carrying
  deg_inv) accumulating feature-major y_raw per 512-dest pack in PSUM;
  the W_l transform (lhsT = y_raw chunk, rhs = W) lands slot-major
  [128 dest, 128 hid] tiles; activation + residual run slot-major
  full-width; the next table is DMA'd out row-major and AllGathered.

  S matrices and the index stream are small (dest-dense cells of
  (pack, src-half)) and stay SBUF-resident across all 4 layers.

Sharding: nodes split across 8 cores by destination (graph parallel);
per-layer AllGather of the fp16 slot-major table; weights replicated.
SPMD: one program for all cores; per-device variation (indices, S
matrices, features) is data. Chunk counts / S spans are made
device-uniform by padding to the cross-device maximum.

Edge indices are int16, so the node table is addressed in two halves
(src position < 32768 vs >= 32768); cells are keyed (pack, half).
"""

import sys

for _p in ("/opt/trn_rl_repo", "/root/.axon_site/_ro/trn_rl_repo"):
    if _p not in sys.path:
        sys.path.insert(0, _p)

from dataclasses import dataclass

import numpy as np

import concourse.bass as bass  # noqa: F401
import concourse.tile as tile
from concourse import bacc, mybir
from concourse.masks import make_identity

P = 128
HALF = 32768  # int16 index range per table half


@dataclass
class Cfg:
    N: int = 50000
    E: int = 800000
    NFEAT: int = 500
    NHID: int = 128
    NCLASS: int = 40
    NLAYERS: int = 4
    GAMMA: float = 0.3
    X1: float = 0.1
    X2: float = 0.9
    C_ACT: float = -1.0
    n_cores: int = 8
    PACK: int = 512

    @property
    def R(self):
        return self.N // self.n_cores          # 6250 dest rows per core

    @property
    def NPACK(self):
        return (self.R + self.PACK - 1) // self.PACK   # 13

    @property
    def NFP(self):
        return ((self.NFEAT + 1 + P - 1) // P) * P     # 512


class Sched:
    """Device-uniform schedule over cells keyed (pack k, half h)."""

    def __init__(self, cfg, counts, spans):
        self.B = counts.max(axis=0)                      # [NPACK, 2]
        self.nch = (self.B + P - 1) // P
        self.cell_off = {}
        off = 0
        for k in range(cfg.NPACK):
            for h in range(2):
                self.cell_off[(k, h)] = off
                off += int(self.nch[k, h]) * P
        self.idx_total = off
        s_off = 0
        self.s_cell = {}
        for k in range(cfg.NPACK):
            for h in range(2):
                metas = []
                for ci in range(int(self.nch[k, h])):
                    co, w = spans[(k, h, ci)]
                    metas.append((co, w, s_off))
                    s_off += w
                self.s_cell[(k, h)] = metas
        self.s_total = s_off


def preprocess(cfg: Cfg, features, edge_row, edge_col, W_in, Ws, c, W_out):
    N, R, ncores = cfg.N, cfg.R, cfg.n_cores
    NPACK, PACK = cfg.NPACK, cfg.PACK
    f32 = np.float32

    deg = np.bincount(edge_row, minlength=N)
    deg_inv = (1.0 / np.maximum(deg, 1)).astype(f32)
    owner = edge_row // R

    # Node permutation: within each device, order nodes by descending global
    # degree so cumulative degree profiles align across devices (smaller S
    # spans, less cell padding). pos[] maps orig node id -> its table
    # position within the owning device's slice.
    dest_of = np.empty((ncores, R), np.int64)   # sorted pos -> orig local
    pos = np.empty(N, np.int64)
    for d in range(ncores):
        order = np.argsort(-deg[d * R:(d + 1) * R], kind="stable")
        dest_of[d] = order
        local_of = np.empty(R, np.int64)
        local_of[order] = np.arange(R)
        pos[d * R:(d + 1) * R] = local_of

    dev = []
    counts = np.zeros((ncores, NPACK, 2), np.int64)
    for d in range(ncores):
        m = owner == d
        dl = pos[edge_row[m]]
        src = edge_col[m].astype(np.int64)
        gp = (src // R) * R + pos[src]          # global table position
        h = gp // HALF                          # table half (0 or 1)
        iv = gp - h * HALF                      # int16-safe row index
        k = dl // PACK
        so = np.lexsort((dl, h, k))
        dl, h, iv, k = dl[so], h[so], iv[so], k[so]
        cell_id = k * 2 + h
        cnt = np.bincount(cell_id, minlength=NPACK * 2)
        counts[d] = cnt.reshape(NPACK, 2)
        dev.append((dl, iv, cell_id))

    B = counts.max(axis=0)
    nch = (B + P - 1) // P

    spans = {}
    dev_cell_start = []
    for d in range(ncores):
        cnt = counts[d].reshape(-1)
        dev_cell_start.append(np.concatenate([[0], np.cumsum(cnt)]))
    for k in range(NPACK):
        for h in range(2):
            cid = k * 2 + h
            for ci in range(int(nch[k, h])):
                fd, ld = [], []
                for d in range(ncores):
                    dl = dev[d][0]
                    b = dev_cell_start[d][cid]
                    n_d = counts[d, k, h]
                    s0 = ci * P
                    if s0 < n_d:
                        s1 = min(s0 + P, n_d)
                        fd.append(int(dl[b + s0]))
                        ld.append(int(dl[b + s1 - 1]))
                co = min(fd) - k * PACK
                w = max(ld) - min(fd) + 1
                spans[(k, h, ci)] = (co, w)

    sch = Sched(cfg, counts, spans)

    # shared weights
    NFP = cfg.NFP
    W_aug = np.zeros((NFP, cfg.NHID), f32)
    W_aug[:cfg.NFEAT] = (1.0 - cfg.GAMMA) * W_in
    W_aug[cfg.NFEAT] = cfg.GAMMA * np.maximum(c, 0.0)
    nk = NFP // P
    W_dram = np.empty((P, nk * P), np.float16)
    for kk in range(nk):
        W_dram[:, kk * P:(kk + 1) * P] = W_aug[kk * P:(kk + 1) * P]
    Ws_dram = np.empty((P, cfg.NLAYERS * P), np.float16)
    for l in range(cfg.NLAYERS):
        Ws_dram[:, l * P:(l + 1) * P] = Ws[l]
    Wout_dram = np.ascontiguousarray(W_out).astype(np.float16)

    in_maps = []
    for d in range(ncores):
        dl, iv, cell_id = dev[d]
        cstart = dev_cell_start[d]
        idx_vals = np.zeros(sch.idx_total, np.int16)
        s_data = np.zeros((P, sch.s_total), np.float16)
        for k in range(NPACK):
            for h in range(2):
                cid = k * 2 + h
                n_d = int(counts[d, k, h])
                if n_d == 0:
                    continue
                b = cstart[cid]
                o = sch.cell_off[(k, h)]
                idx_vals[o:o + n_d] = iv[b:b + n_d].astype(np.int16)
                metas = sch.s_cell[(k, h)]
                sl = np.arange(n_d)
                ci_arr = sl // P
                row = sl % P
                co = np.array([m[0] for m in metas], np.int64)[ci_arr]
                soff = np.array([m[2] for m in metas], np.int64)[ci_arr]
                col = soff + (dl[b:b + n_d] - k * PACK - co)
                s_data[row, col] = deg_inv[
                    d * R + dest_of[d][dl[b:b + n_d]]]
        idx_t = np.tile(idx_vals.reshape(-1, 16).T, (8, 1))

        gids = d * R + dest_of[d]
        featT = np.zeros((NFP, R), np.float16)
        featT[:cfg.NFEAT] = features[gids].T
        featT[cfg.NFEAT] = 1.0

        in_maps.append(dict(
            featT=featT, idx_all=np.ascontiguousarray(idx_t), s_all=s_data,
            w_proj=W_dram, w_hid=Ws_dram, w_out=Wout_dram,
        ))
    return in_maps, sch, dest_of


def build_program(cfg: Cfg, sch: Sched):
    nc = bacc.Bacc("TRN2", target_bir_lowering=False, debug=False,
                   num_devices=cfg.n_cores, num_swdge_queues=2)
    DT = mybir.dt.float16
    f32 = mybir.dt.float32
    R, NPACK, NFP = cfg.R, cfg.NPACK, cfg.NFP
    AFT = mybir.ActivationFunctionType
    ALU = mybir.AluOpType
    AX = mybir.AxisListType
    RED = bass.bass_isa.ReduceOp
    rg = [list(range(cfg.n_cores))]
    nk = NFP // P
    NCHK = (R + cfg.PACK - 1) // cfg.PACK   # 512-col chunks over R
    NT = R // P + (1 if R % P else 0)       # 128-col tiles over R (49)

    def cw(ch):
        return min(cfg.PACK, R - ch * cfg.PACK)

    featT = nc.dram_tensor("featT", [NFP, R], DT, kind="ExternalInput").ap()
    idx_all = nc.dram_tensor("idx_all", [P, sch.idx_total // 16],
                             mybir.dt.int16, kind="ExternalInput").ap()
    s_all = nc.dram_tensor("s_all", [P, sch.s_total], DT,
                           kind="ExternalInput").ap()
    w_proj = nc.dram_tensor("w_proj", [P, nk * P], DT,
                            kind="ExternalInput").ap()
    w_hid = nc.dram_tensor("w_hid", [P, cfg.NLAYERS * P], DT,
                           kind="ExternalInput").ap()
    w_out = nc.dram_tensor("w_out", [P, cfg.NCLASS], DT,
                           kind="ExternalInput").ap()
    out = nc.dram_tensor("out", [R, cfg.NCLASS], f32,
                         kind="ExternalOutput").ap()

    INV08 = float(np.float32(1.0 / (np.float64(cfg.X2) - cfg.X1 + 1e-8)))
    B_RELU = float(np.float32(-cfg.X1 * INV08))
    E1 = float(1.0 + np.exp(-cfg.C_ACT))
    NIT = cfg.NLAYERS
    FT = R // P          # full 128-row tiles (48)
    TAIL = R - FT * P    # ragged tail rows (106)
    SMW = NT * P         # slot-major width (6272)

    # gather calls: group cells into calls of <= GCAP slots (contiguous in
    # the idx stream). Each call is one dma_gather against one table half.
    # GCAP is bounded by the SWDGE descriptor-ring carveout: one call's
    # descriptors (num_idxs/16 + 1 per DMA engine) must fit the ring or the
    # Q7 waits forever (hardware-verified: 1024 ok, 2560 wedges).
    GCAP = 896
    calls = []     # (half, slot_off, n_slots)
    for k in range(NPACK):
        for h in range(2):
            n = int(sch.nch[k, h]) * P
            off = sch.cell_off[(k, h)]
            while n > 0:
                take = min(n, GCAP)
                # merge with previous call if same half and contiguous
                if calls and calls[-1][0] == h and \
                        calls[-1][1] + calls[-1][2] == off and \
                        calls[-1][2] + take <= GCAP:
                    calls[-1] = (h, calls[-1][1], calls[-1][2] + take)
                else:
                    calls.append((h, off, take))
                off += take
                n -= take
    # map chunk -> (call index, chunk offset within call)
    chunk_call = {}
    for idx_c, (h, off, n) in enumerate(calls):
        for j in range(n // P):
            chunk_call[off + j * P] = (idx_c, j)

    with tile.TileContext(nc) as tc:
        with tc.tile_pool(name="persist", bufs=1) as persist, \
             tc.tile_pool(name="dram", bufs=1, space="DRAM") as dram:
            idx_sb = persist.tile([P, sch.idx_total // 16], mybir.dt.int16)
            nc.sync.dma_start(idx_sb[:], idx_all[:])
            s_sb = persist.tile([P, sch.s_total], DT)
            nc.scalar.dma_start(s_sb[:], s_all[:])
            wh_sb = persist.tile([P, cfg.NLAYERS * P], DT)
            nc.sync.dma_start(wh_sb[:], w_hid[:])
            wo_sb = persist.tile([P, cfg.NCLASS], DT)
            nc.sync.dma_start(wo_sb[:], w_out[:])
            wp_sb = persist.tile([P, nk * P], DT)
            nc.sync.dma_start(wp_sb[:], w_proj[:])
            ones1 = persist.tile([1, P], f32)
            nc.vector.memset(ones1[:], 1.0)
            b_relu = persist.tile([P, 1], f32)
            nc.vector.memset(b_relu[:], B_RELU)
            idn = persist.tile([P, P], DT)
            make_identity(nc, idn[:])
            mm_sb = persist.tile([P, 2], f32)
            mm_ar = persist.tile([P, 2], f32)
            mm_back = persist.tile([1, 2], f32)
            sfac = persist.tile([P, 1], f32)
            bfac = persist.tile([P, 1], f32)
            x0sm = persist.tile([P, SMW], DT)   # slot-major x0 (beta-scaled)
            a1 = persist.tile([P, SMW], DT)     # slot-major activation buf
            # tail-tile pad lanes (partitions >= TAIL of the last 128-col
            # block) are never written by the tiled producers but are read
            # by full-width elementwise ops — zero them once.
            nc.vector.memset(x0sm[:, FT * P:], 0.0)
            nc.vector.memset(a1[:, FT * P:], 0.0)
            xact = persist.tile([P, SMW], DT)   # sigmoid scratch
            xn16 = persist.tile([P, SMW], DT)   # slot-major layer output

            def dma_slot_major_out(eng, bnc, src):
                # src slot-major [P, SMW] -> bnc row-major [R, P]
                eng.dma_start(
                    bnc[:FT * P, :].rearrange("(t p) h -> p t h", p=P),
                    src[:, :FT * P].rearrange("p (t h) -> p t h", h=P))
                eng.dma_start(bnc[FT * P:R, :], src[:TAIL, FT * P:FT * P + P])

            bounce = [dram.tile([R, P], DT, name=f"bounce{i}")
                      for i in range(NIT)]
            x_full = [dram.tile([cfg.N, P], DT, addr_space="Shared",
                                name=f"x_full{i}") for i in range(NIT)]
            mm_in = dram.tile([1, 2], f32)
            mm_out = dram.tile([1, 2], f32, addr_space="Shared")

            # ================= projection =================
            with tc.tile_pool(name="strips", bufs=1) as strip_pool, \
                 tc.tile_pool(name="pwork", bufs=2) as pwork, \
                 tc.tile_pool(name="pps", bufs=2, space="PSUM") as pps_pool, \
                 tc.tile_pool(name="tps", bufs=4, space="PSUM") as tps_pool:
                strips = []
                for k in range(nk):
                    st = strip_pool.tile([P, R], DT, name=f"strip{k}",
                                         tag=f"strip{k}")
                    nc.sync.dma_start(st[:], featT[k * P:(k + 1) * P, :])
                    strips.append(st)
                x0T = strip_pool.tile([P, R], DT, name="x0T", tag="x0T")
                yT = strip_pool.tile([P, R], f32, name="yT", tag="yT")
                rmax = pwork.tile([P, 1], f32, name="rmax", tag="rmax")
                rmin = pwork.tile([P, 1], f32, name="rmin", tag="rmin")
                for ch in range(NCHK):
                    w = cw(ch)
                    sl = slice(ch * cfg.PACK, ch * cfg.PACK + w)
                    ps = pps_pool.tile([P, cfg.PACK], f32, name=f"h{ch}",
                                       tag="hps")
                    for k in range(nk):
                        nc.tensor.matmul(ps[:, :w],
                                         lhsT=wp_sb[:, k * P:(k + 1) * P],
                                         rhs=strips[k][:, sl],
                                         start=(k == 0), stop=(k == nk - 1))
                    nc.vector.tensor_copy(yT[:, sl], ps[:, :w])
                    qmax = pwork.tile([P, 1], f32, name="qmax", tag="qmax")
                    qmin = pwork.tile([P, 1], f32, name="qmin", tag="qmin")
                    nc.vector.tensor_reduce(qmax[:], ps[:, :w], axis=AX.X,
                                            op=ALU.max)
                    nc.vector.tensor_reduce(qmin[:], ps[:, :w], axis=AX.X,
                                            op=ALU.min)
                    if ch == 0:
                        nc.vector.tensor_copy(rmax[:], qmax[:])
                        nc.vector.tensor_copy(rmin[:], qmin[:])
                    else:
                        nc.vector.tensor_tensor(rmax[:], rmax[:], qmax[:],
                                                op=ALU.max)
                        nc.vector.tensor_tensor(rmin[:], rmin[:], qmin[:],
                                                op=ALU.min)
                nc.vector.tensor_copy(mm_sb[:, 0:1], rmax[:])
                nc.vector.tensor_scalar(mm_sb[:, 1:2], rmin[:], -1.0, None,
                                        ALU.mult)
                nc.gpsimd.partition_all_reduce(mm_ar[:], mm_sb[:],
                                               channels=P, reduce_op=RED.max)
                nc.sync.dma_start(mm_in[:], mm_ar[0:1, :])
                nc.gpsimd.collective_compute(
                    "AllReduce", ALU.max, ins=[mm_in.opt()],
                    outs=[mm_out.opt()], replica_groups=rg)
                nc.sync.dma_start(mm_back[:], mm_out[:])
                bc_ps = pps_pool.tile([P, 2], f32, name="bc_ps", tag="hps")
                nc.tensor.matmul(bc_ps[:], lhsT=ones1[:], rhs=mm_back[:],
                                 start=True, stop=True)
                bcast = pwork.tile([P, 2], f32, name="bcast", tag="qmin")
                nc.vector.tensor_copy(bcast[:], bc_ps[:])
                sden = pwork.tile([P, 1], f32, name="sden", tag="qmax")
                nc.vector.tensor_tensor(sden[:], bcast[:, 0:1], bcast[:, 1:2],
                                        op=ALU.add)
                nc.vector.tensor_scalar(sden[:], sden[:], 1e-8, None, ALU.add)
                nc.vector.reciprocal(sfac[:], sden[:])
                nc.vector.tensor_tensor(bfac[:], bcast[:, 1:2], sfac[:],
                                        op=ALU.mult)
                nc.vector.tensor_scalar(x0T[:], yT[:], sfac[:], bfac[:],
                                        ALU.mult, ALU.add)
                # transpose x0T (feature-major) -> x0sm (slot-major)
                for t in range(NT):
                    w = min(P, R - t * P)
                    tp = tps_pool.tile([P, P], f32, name=f"x0t{t}", tag="tps")
                    nc.tensor.matmul(tp[:w, :], lhsT=x0T[:, t * P:t * P + w],
                                     rhs=idn[:], start=True, stop=True)
                    nc.scalar.activation(x0sm[:w, t * P:(t + 1) * P],
                                         tp[:w, :], AFT.Copy)
                dma_slot_major_out(nc.sync, bounce[0], x0sm)
            nc.gpsimd.collective_compute(
                "AllGather", ALU.bypass, ins=[bounce[0].opt()],
                outs=[x_full[0].opt()], replica_groups=rg)

            # ================= conv layers =================
            with tc.tile_pool(name="gpool", bufs=10) as gpool, \
                 tc.tile_pool(name="ywork", bufs=3) as ywork, \
                 tc.tile_pool(name="yps", bufs=3, space="PSUM") as yps_pool, \
                 tc.tile_pool(name="tps2", bufs=4, space="PSUM") as tps2_pool:
                beta_prev = 1.0
                for l in range(NIT):
                    last = l == NIT - 1
                    beta = min(0.5, (l + 1) / cfg.NLAYERS * 0.5)
                    c1 = float((1.0 - beta) * E1)
                    nc.vector.tensor_scalar(x0sm[:], x0sm[:],
                                            float(beta / beta_prev), None,
                                            ALU.mult)
                    beta_prev = beta
                    xf = x_full[l][:]
                    halves = [xf[:HALF, :], xf[HALF:, :]]
                    gts = []
                    for (h, off, n) in calls:
                        gt = gpool.tile([P, GCAP], DT,
                                        name=f"g{l}_{off}", tag="g")
                        nc.gpsimd.dma_gather(
                            gt[:, :n].rearrange("p (n e) -> p n e", e=P),
                            halves[h],
                            idx_sb[:, off // 16:(off + n) // 16],
                            num_idxs=n, num_idxs_reg=n,
                            elem_size=P, queue_num=len(gts) % 2)
                        gts.append(gt)
                    for k in range(NPACK):
                        wk = cw(k)
                        n_mm = sum(len(sch.s_cell[(k, h)]) for h in range(2))
                        ps = yps_pool.tile([P, cfg.PACK], f32,
                                           name=f"y{l}_{k}", tag="yps")
                        nc.vector.memset(ps[:, :wk], 0.0)
                        mi = 0
                        for h in range(2):
                            base = sch.cell_off[(k, h)]
                            for ci, (co, wdt, soff) in enumerate(
                                    sch.s_cell[(k, h)]):
                                cidx, j = chunk_call[base + ci * P]
                                lv = gts[cidx][:, j * P:(j + 1) * P]
                                wdt2 = min(wdt, wk - co)
                                nc.tensor.matmul(
                                    ps[:, co:co + wdt2],
                                    lhsT=lv,
                                    rhs=s_sb[:, soff:soff + wdt2],
                                    start=False, stop=(mi == n_mm - 1),
                                    skip_group_check=True)
                                mi += 1
                        yraw = ywork.tile([P, cfg.PACK], DT,
                                          name=f"yr{l}_{k}", tag="yr")
                        nc.vector.tensor_copy(yraw[:, :wk], ps[:, :wk])
                        # transform + transpose: per 128-dest chunk
                        for c in range((wk + P - 1) // P):
                            w = min(P, wk - c * P)
                            gcol = k * cfg.PACK + c * P
                            tp = tps2_pool.tile([P, P], f32,
                                                name=f"t{l}_{k}_{c}",
                                                tag="tps2")
                            nc.tensor.matmul(
                                tp[:w, :], lhsT=yraw[:, c * P:c * P + w],
                                rhs=wh_sb[:, l * P:(l + 1) * P],
                                start=True, stop=True)
                            nc.scalar.activation(
                                a1[:w, gcol:gcol + P], tp[:w, :], AFT.Relu,
                                bias=b_relu[:w], scale=INV08)
                    # activation chain (slot-major, full width, fp16)
                    nc.vector.tensor_scalar(a1[:], a1[:], 1.0, c1,
                                            ALU.min, ALU.mult)
                    nc.scalar.activation(xact[:], a1[:], AFT.Sigmoid,
                                         scale=float(-1.0 / c1))
                    nc.vector.tensor_tensor(a1[:], a1[:], xact[:],
                                            op=ALU.mult)
                    nc.vector.tensor_tensor(xn16[:], a1[:], x0sm[:],
                                            op=ALU.add)
                    if not last:
                        dma_slot_major_out(nc.sync, bounce[l + 1], xn16)
                        nc.gpsimd.collective_compute(
                            "AllGather", ALU.bypass, ins=[bounce[l + 1].opt()],
                            outs=[x_full[l + 1].opt()], replica_groups=rg)

            # ================= output stage =================
            with tc.tile_pool(name="ow", bufs=2) as ow, \
                 tc.tile_pool(name="ops", bufs=2, space="PSUM") as ops_pool:
                for t in range(NT):
                    w = min(P, R - t * P)
                    tp = ops_pool.tile([P, P], f32, name=f"xt{t}", tag="oxt")
                    nc.tensor.matmul(
                        tp[:, :w],
                        lhsT=xn16[:w, t * P:(t + 1) * P],
                        rhs=idn[:w, :w], start=True, stop=True)
                    xnT = ow.tile([P, P], DT, name="xnT", tag="xnT")
                    nc.scalar.activation(xnT[:, :w], tp[:, :w], AFT.Copy)
                    lg = ops_pool.tile([P, cfg.NCLASS], f32,
                                       name=f"lg{t}", tag="olg")
                    nc.tensor.matmul(lg[:w, :], lhsT=xnT[:, :w],
                                     rhs=wo_sb[:], start=True, stop=True)
                    mx = ow.tile([P, 1], f32, name="mx", tag="mx")
                    nc.vector.tensor_reduce(mx[:w], lg[:w, :],
                                            axis=AX.X, op=ALU.max)
                    nmx = ow.tile([P, 1], f32, name="nmx", tag="nmx")
                    nc.vector.tensor_scalar(nmx[:w], mx[:w], -1.0,
                                            None, ALU.mult)
                    ex = ow.tile([P, cfg.NCLASS], f32, name="ex", tag="ex")
                    sume = ow.tile([P, 1], f32, name="sume", tag="sume")
                    nc.scalar.activation(ex[:w], lg[:w, :], AFT.Exp,
                                         bias=nmx[:w], accum_out=sume[:w])
                    lse = ow.tile([P, 1], f32, name="lse", tag="lse")
                    nc.scalar.activation(lse[:w], sume[:w], AFT.Ln)
                    nc.vector.tensor_tensor(lse[:w], lse[:w], nmx[:w],
                                            op=ALU.subtract)
                    res = ow.tile([P, cfg.NCLASS], f32, name="res", tag="ex")
                    nc.vector.tensor_scalar(res[:w], lg[:w, :], lse[:w],
                                            None, ALU.subtract)
                    nc.sync.dma_start(out[t * P:t * P + w, :], res[:w])
    nc.compile()
    return nc


def kernel(**inputs) -> np.ndarray:
    cfg = Cfg()
    features = np.asarray(inputs["features"], np.float32)
    edge_row = np.asarray(inputs["edge_row"], np.int64)
    edge_col = np.asarray(inputs["edge_col"], np.int64)
    W_in = np.asarray(inputs["W_in"], np.float32)
    Ws = np.asarray(inputs["Ws"], np.float32)
    c = np.asarray(inputs["c"], np.float32)
    W_out = np.asarray(inputs["W_out"], np.float32)

    in_maps, sch, dest_of = preprocess(cfg, features, edge_row, edge_col,
                                       W_in, Ws, c, W_out)
    nc = build_program(cfg, sch)

    import os
    from concourse import bass_utils
    res = bass_utils.run_bass_kernel_spmd(
        nc, in_maps, core_ids=list(range(cfg.n_cores)),
        trace=bool(os.environ.get("GNN_TRACE")))
    kernel.last_result = res
    out = np.empty((cfg.N, cfg.NCLASS), np.float32)
    for d in range(cfg.n_cores):
        out[d * cfg.R + dest_of[d]] = res.results[d]["out"]
    return out


# revision 14
# speedup vs baseline: 8.6356x; 1.3833x over previous
"""Trainium2 Bass kernel for nn_NewActivationGNN (GNN message passing).

Architecture (v2, swdge gather):
  y_l = A_norm @ (x @ W_l) is computed as (A_norm @ x) @ W_l — aggregate
  raw hidden rows first, transform after. Per-edge source rows are pulled
  straight from the DRAM-resident AllGathered node table with SWDGE
  dma_gather (one 256B descriptor per edge, slot-major destination), so
  the gather costs ~23ns/edge across 16 SDMA engines instead of ~26ns/edge
  serialized on gpsimd Q7 cores.

  Slot-major pipeline: gathered chunks [128 edge-slots, 128 hid] are lhsT
  for selection-matrix matmuls (rhs = S fp16 [slot, dest-span] carrying
  deg_inv) accumulating feature-major y_raw per 512-dest pack in PSUM;
  the W_l transform (lhsT = y_raw chunk, rhs = W) lands slot-major
  [128 dest, 128 hid] tiles; activation + residual run slot-major
  full-width; the next table is DMA'd out row-major and AllGathered.

  S matrices and the index stream are small (dest-dense cells of
  (pack, src-half)) and stay SBUF-resident across all 4 layers.

Sharding: nodes split across 8 cores by destination (graph parallel);
per-layer AllGather of the fp16 slot-major table; weights replicated.
SPMD: one program for all cores; per-device variation (indices, S
matrices, features) is data. Chunk counts / S spans are made
device-uniform by padding to the cross-device maximum.

Edge indices are int16, so the node table is addressed in two halves
(src position < 32768 vs >= 32768); cells are keyed (pack, half).
"""

import sys

for _p in ("/opt/trn_rl_repo", "/root/.axon_site/_ro/trn_rl_repo"):
    if _p not in sys.path:
        sys.path.insert(0, _p)

from dataclasses import dataclass

import numpy as np

import concourse.bass as bass  # noqa: F401
import concourse.tile as tile
from concourse import bacc, mybir
from concourse.masks import make_identity

P = 128
HALF = 32768  # int16 index range per table half


@dataclass
class Cfg:
    N: int = 50000
    E: int = 800000
    NFEAT: int = 500
    NHID: int = 128
    NCLASS: int = 40
    NLAYERS: int = 4
    GAMMA: float = 0.3
    X1: float = 0.1
    X2: float = 0.9
    C_ACT: float = -1.0
    n_cores: int = 8
    PACK: int = 512

    @property
    def R(self):
        return self.N // self.n_cores          # 6250 dest rows per core

    @property
    def NPACK(self):
        return (self.R + self.PACK - 1) // self.PACK   # 13

    @property
    def NFP(self):
        return ((self.NFEAT + 1 + P - 1) // P) * P     # 512


class Sched:
    """Device-uniform schedule over cells keyed (pack k, half h)."""

    def __init__(self, cfg, counts, spans):
        self.B = counts.max(axis=0)                      # [NPACK, 2]
        self.nch = (self.B + P - 1) // P
        self.cell_off = {}
        off = 0
        for k in range(cfg.NPACK):
            for h in range(2):
                self.cell_off[(k, h)] = off
                off += int(self.nch[k, h]) * P
        self.idx_total = off
        s_off = 0
        self.s_cell = {}
        for k in range(cfg.NPACK):
            for h in range(2):
                metas = []
                for ci in range(int(self.nch[k, h])):
                    co, w = spans[(k, h, ci)]
                    metas.append((co, w, s_off))
                    s_off += w
                self.s_cell[(k, h)] = metas
        self.s_total = s_off


def preprocess(cfg: Cfg, features, edge_row, edge_col, W_in, Ws, c, W_out):
    N, R, ncores = cfg.N, cfg.R, cfg.n_cores
    NPACK, PACK = cfg.NPACK, cfg.PACK
    f32 = np.float32

    deg = np.bincount(edge_row, minlength=N)
    deg_inv = (1.0 / np.maximum(deg, 1)).astype(f32)
    owner = edge_row // R

    # Node permutation: within each device, order nodes by descending global
    # degree so cumulative degree profiles align across devices (smaller S
    # spans, less cell padding). pos[] maps orig node id -> its table
    # position within the owning device's slice.
    dest_of = np.empty((ncores, R), np.int64)   # sorted pos -> orig local
    pos = np.empty(N, np.int64)
    for d in range(ncores):
        order = np.argsort(-deg[d * R:(d + 1) * R], kind="stable")
        dest_of[d] = order
        local_of = np.empty(R, np.int64)
        local_of[order] = np.arange(R)
        pos[d * R:(d + 1) * R] = local_of

    dev = []
    counts = np.zeros((ncores, NPACK, 2), np.int64)
    for d in range(ncores):
        m = owner == d
        dl = pos[edge_row[m]]
        src = edge_col[m].astype(np.int64)
        gp = (src // R) * R + pos[src]          # global table position
        h = gp // HALF                          # table half (0 or 1)
        iv = gp - h * HALF                      # int16-safe row index
        k = dl // PACK
        so = np.lexsort((dl, h, k))
        dl, h, iv, k = dl[so], h[so], iv[so], k[so]
        cell_id = k * 2 + h
        cnt = np.bincount(cell_id, minlength=NPACK * 2)
        counts[d] = cnt.reshape(NPACK, 2)
        dev.append((dl, iv, cell_id))

    B = counts.max(axis=0)
    nch = (B + P - 1) // P

    spans = {}
    dev_cell_start = []
    for d in range(ncores):
        cnt = counts[d].reshape(-1)
        dev_cell_start.append(np.concatenate([[0], np.cumsum(cnt)]))
    for k in range(NPACK):
        for h in range(2):
            cid = k * 2 + h
            for ci in range(int(nch[k, h])):
                fd, ld = [], []
                for d in range(ncores):
                    dl = dev[d][0]
                    b = dev_cell_start[d][cid]
                    n_d = counts[d, k, h]
                    s0 = ci * P
                    if s0 < n_d:
                        s1 = min(s0 + P, n_d)
                        fd.append(int(dl[b + s0]))
                        ld.append(int(dl[b + s1 - 1]))
                co = min(fd) - k * PACK
                w = max(ld) - min(fd) + 1
                spans[(k, h, ci)] = (co, w)

    sch = Sched(cfg, counts, spans)

    # shared weights
    NFP = cfg.NFP
    W_aug = np.zeros((NFP, cfg.NHID), f32)
    W_aug[:cfg.NFEAT] = (1.0 - cfg.GAMMA) * W_in
    W_aug[cfg.NFEAT] = cfg.GAMMA * np.maximum(c, 0.0)
    nk = NFP // P
    W_dram = np.empty((P, nk * P), np.float16)
    for kk in range(nk):
        W_dram[:, kk * P:(kk + 1) * P] = W_aug[kk * P:(kk + 1) * P]
    Ws_dram = np.empty((P, cfg.NLAYERS * P), np.float16)
    for l in range(cfg.NLAYERS):
        Ws_dram[:, l * P:(l + 1) * P] = Ws[l]
    Wout_dram = np.ascontiguousarray(W_out).astype(np.float16)

    in_maps = []
    for d in range(ncores):
        dl, iv, cell_id = dev[d]
        cstart = dev_cell_start[d]
        idx_vals = np.zeros(sch.idx_total, np.int16)
        s_data = np.zeros((P, sch.s_total), np.float16)
        for k in range(NPACK):
            for h in range(2):
                cid = k * 2 + h
                n_d = int(counts[d, k, h])
                if n_d == 0:
                    continue
                b = cstart[cid]
                o = sch.cell_off[(k, h)]
                idx_vals[o:o + n_d] = iv[b:b + n_d].astype(np.int16)
                metas = sch.s_cell[(k, h)]
                sl = np.arange(n_d)
                ci_arr = sl // P
                row = sl % P
                co = np.array([m[0] for m in metas], np.int64)[ci_arr]
                soff = np.array([m[2] for m in metas], np.int64)[ci_arr]
                col = soff + (dl[b:b + n_d] - k * PACK - co)
                s_data[row, col] = deg_inv[
                    d * R + dest_of[d][dl[b:b + n_d]]]
        idx_t = np.tile(idx_vals.reshape(-1, 16).T, (8, 1))

        gids = d * R + dest_of[d]
        featT = np.zeros((NFP, R), np.float16)
        featT[:cfg.NFEAT] = features[gids].T
        featT[cfg.NFEAT] = 1.0

        in_maps.append(dict(
            featT=featT, idx_all=np.ascontiguousarray(idx_t), s_all=s_data,
            w_proj=W_dram, w_hid=Ws_dram, w_out=Wout_dram,
        ))
    return in_maps, sch, dest_of


def build_program(cfg: Cfg, sch: Sched):
    nc = bacc.Bacc("TRN2", target_bir_lowering=False, debug=False,
                   num_devices=cfg.n_cores, num_swdge_queues=4)
    DT = mybir.dt.float16
    f32 = mybir.dt.float32
    R, NPACK, NFP = cfg.R, cfg.NPACK, cfg.NFP
    AFT = mybir.ActivationFunctionType
    ALU = mybir.AluOpType
    AX = mybir.AxisListType
    RED = bass.bass_isa.ReduceOp
    rg = [list(range(cfg.n_cores))]
    nk = NFP // P
    NCHK = (R + cfg.PACK - 1) // cfg.PACK   # 512-col chunks over R
    NT = R // P + (1 if R % P else 0)       # 128-col tiles over R (49)

    def cw(ch):
        return min(cfg.PACK, R - ch * cfg.PACK)

    featT = nc.dram_tensor("featT", [NFP, R], DT, kind="ExternalInput").ap()
    idx_all = nc.dram_tensor("idx_all", [P, sch.idx_total // 16],
                             mybir.dt.int16, kind="ExternalInput").ap()
    s_all = nc.dram_tensor("s_all", [P, sch.s_total], DT,
                           kind="ExternalInput").ap()
    w_proj = nc.dram_tensor("w_proj", [P, nk * P], DT,
                            kind="ExternalInput").ap()
    w_hid = nc.dram_tensor("w_hid", [P, cfg.NLAYERS * P], DT,
                           kind="ExternalInput").ap()
    w_out = nc.dram_tensor("w_out", [P, cfg.NCLASS], DT,
                           kind="ExternalInput").ap()
    out = nc.dram_tensor("out", [R, cfg.NCLASS], f32,
                         kind="ExternalOutput").ap()

    INV08 = float(np.float32(1.0 / (np.float64(cfg.X2) - cfg.X1 + 1e-8)))
    B_RELU = float(np.float32(-cfg.X1 * INV08))
    E1 = float(1.0 + np.exp(-cfg.C_ACT))
    NIT = cfg.NLAYERS
    FT = R // P          # full 128-row tiles (48)
    TAIL = R - FT * P    # ragged tail rows (106)
    SMW = NT * P         # slot-major width (6272)

    # gather calls: group cells into calls of <= GCAP slots (contiguous in
    # the idx stream). Each call is one dma_gather against one table half.
    # GCAP is bounded by the SWDGE descriptor-ring carveout: one call's
    # descriptors (num_idxs/16 + 1 per DMA engine) must fit the ring or the
    # Q7 waits forever (hardware-verified: 1024 ok, 2560 wedges).
    GCAP = 896
    calls = []     # (half, slot_off, n_slots)
    for k in range(NPACK):
        for h in range(2):
            n = int(sch.nch[k, h]) * P
            off = sch.cell_off[(k, h)]
            while n > 0:
                take = min(n, GCAP)
                # merge with previous call if same half and contiguous
                if calls and calls[-1][0] == h and \
                        calls[-1][1] + calls[-1][2] == off and \
                        calls[-1][2] + take <= GCAP:
                    calls[-1] = (h, calls[-1][1], calls[-1][2] + take)
                else:
                    calls.append((h, off, take))
                off += take
                n -= take
    # map chunk -> (call index, chunk offset within call)
    chunk_call = {}
    for idx_c, (h, off, n) in enumerate(calls):
        for j in range(n // P):
            chunk_call[off + j * P] = (idx_c, j)

    with tile.TileContext(nc) as tc:
        with tc.tile_pool(name="persist", bufs=1) as persist, \
             tc.tile_pool(name="dram", bufs=1, space="DRAM") as dram:
            idx_sb = persist.tile([P, sch.idx_total // 16], mybir.dt.int16)
            nc.sync.dma_start(idx_sb[:], idx_all[:])
            s_sb = persist.tile([P, sch.s_total], DT)
            nc.scalar.dma_start(s_sb[:], s_all[:])
            wh_sb = persist.tile([P, cfg.NLAYERS * P], DT)
            nc.sync.dma_start(wh_sb[:], w_hid[:])
            wo_sb = persist.tile([P, cfg.NCLASS], DT)
            nc.sync.dma_start(wo_sb[:], w_out[:])
            wp_sb = persist.tile([P, nk * P], DT)
            nc.sync.dma_start(wp_sb[:], w_proj[:])
            ones1 = persist.tile([1, P], f32)
            nc.vector.memset(ones1[:], 1.0)
            b_relu = persist.tile([P, 1], f32)
            nc.vector.memset(b_relu[:], B_RELU)
            idn = persist.tile([P, P], DT)
            make_identity(nc, idn[:])
            mm_sb = persist.tile([P, 2], f32)
            mm_ar = persist.tile([P, 2], f32)
            mm_back = persist.tile([1, 2], f32)
            sfac = persist.tile([P, 1], f32)
            bfac = persist.tile([P, 1], f32)
            x0sm = persist.tile([P, SMW], DT)   # slot-major x0 (beta-scaled)
            a1 = persist.tile([P, SMW], DT)     # slot-major activation buf
            # tail-tile pad lanes (partitions >= TAIL of the last 128-col
            # block) are never written by the tiled producers but are read
            # by full-width elementwise ops — zero them once.
            nc.vector.memset(x0sm[:, FT * P:], 0.0)
            nc.vector.memset(a1[:, FT * P:], 0.0)
            xact = persist.tile([P, SMW], DT)   # sigmoid scratch
            xn16 = persist.tile([P, SMW], DT)   # slot-major layer output

            def dma_slot_major_out(eng, bnc, src):
                # src slot-major [P, SMW] -> bnc row-major [R, P]
                eng.dma_start(
                    bnc[:FT * P, :].rearrange("(t p) h -> p t h", p=P),
                    src[:, :FT * P].rearrange("p (t h) -> p t h", h=P))
                eng.dma_start(bnc[FT * P:R, :], src[:TAIL, FT * P:FT * P + P])

            bounce = [dram.tile([R, P], DT, name=f"bounce{i}")
                      for i in range(NIT)]
            x_full = [dram.tile([cfg.N, P], DT, addr_space="Shared",
                                name=f"x_full{i}") for i in range(NIT)]
            mm_in = dram.tile([1, 2], f32)
            mm_out = dram.tile([1, 2], f32, addr_space="Shared")

            # ================= projection =================
            with tc.tile_pool(name="strips", bufs=1) as strip_pool, \
                 tc.tile_pool(name="pwork", bufs=2) as pwork, \
                 tc.tile_pool(name="pps", bufs=2, space="PSUM") as pps_pool, \
                 tc.tile_pool(name="tps", bufs=4, space="PSUM") as tps_pool:
                strips = []
                for k in range(nk):
                    st = strip_pool.tile([P, R], DT, name=f"strip{k}",
                                         tag=f"strip{k}")
                    nc.sync.dma_start(st[:], featT[k * P:(k + 1) * P, :])
                    strips.append(st)
                x0T = strip_pool.tile([P, R], DT, name="x0T", tag="x0T")
                yT = strip_pool.tile([P, R], f32, name="yT", tag="yT")
                rmax = pwork.tile([P, 1], f32, name="rmax", tag="rmax")
                rmin = pwork.tile([P, 1], f32, name="rmin", tag="rmin")
                for ch in range(NCHK):
                    w = cw(ch)
                    sl = slice(ch * cfg.PACK, ch * cfg.PACK + w)
                    ps = pps_pool.tile([P, cfg.PACK], f32, name=f"h{ch}",
                                       tag="hps")
                    for k in range(nk):
                        nc.tensor.matmul(ps[:, :w],
                                         lhsT=wp_sb[:, k * P:(k + 1) * P],
                                         rhs=strips[k][:, sl],
                                         start=(k == 0), stop=(k == nk - 1))
                    nc.vector.tensor_copy(yT[:, sl], ps[:, :w])
                    qmax = pwork.tile([P, 1], f32, name="qmax", tag="qmax")
                    qmin = pwork.tile([P, 1], f32, name="qmin", tag="qmin")
                    nc.vector.tensor_reduce(qmax[:], ps[:, :w], axis=AX.X,
                                            op=ALU.max)
                    nc.vector.tensor_reduce(qmin[:], ps[:, :w], axis=AX.X,
                                            op=ALU.min)
                    if ch == 0:
                        nc.vector.tensor_copy(rmax[:], qmax[:])
                        nc.vector.tensor_copy(rmin[:], qmin[:])
                    else:
                        nc.vector.tensor_tensor(rmax[:], rmax[:], qmax[:],
                                                op=ALU.max)
                        nc.vector.tensor_tensor(rmin[:], rmin[:], qmin[:],
                                                op=ALU.min)
                nc.vector.tensor_copy(mm_sb[:, 0:1], rmax[:])
                nc.vector.tensor_scalar(mm_sb[:, 1:2], rmin[:], -1.0, None,
                                        ALU.mult)
                nc.gpsimd.partition_all_reduce(mm_ar[:], mm_sb[:],
                                               channels=P, reduce_op=RED.max)
                nc.sync.dma_start(mm_in[:], mm_ar[0:1, :])
                nc.gpsimd.collective_compute(
                    "AllReduce", ALU.max, ins=[mm_in.opt()],
                    outs=[mm_out.opt()], replica_groups=rg)
                nc.sync.dma_start(mm_back[:], mm_out[:])
                bc_ps = pps_pool.tile([P, 2], f32, name="bc_ps", tag="hps")
                nc.tensor.matmul(bc_ps[:], lhsT=ones1[:], rhs=mm_back[:],
                                 start=True, stop=True)
                bcast = pwork.tile([P, 2], f32, name="bcast", tag="qmin")
                nc.vector.tensor_copy(bcast[:], bc_ps[:])
                sden = pwork.tile([P, 1], f32, name="sden", tag="qmax")
                nc.vector.tensor_tensor(sden[:], bcast[:, 0:1], bcast[:, 1:2],
                                        op=ALU.add)
                nc.vector.tensor_scalar(sden[:], sden[:], 1e-8, None, ALU.add)
                nc.vector.reciprocal(sfac[:], sden[:])
                nc.vector.tensor_tensor(bfac[:], bcast[:, 1:2], sfac[:],
                                        op=ALU.mult)
                nc.vector.tensor_scalar(x0T[:], yT[:], sfac[:], bfac[:],
                                        ALU.mult, ALU.add)
                # transpose x0T (feature-major) -> x0sm (slot-major)
                for t in range(NT):
                    w = min(P, R - t * P)
                    tp = tps_pool.tile([P, P], f32, name=f"x0t{t}", tag="tps")
                    nc.tensor.matmul(tp[:w, :], lhsT=x0T[:, t * P:t * P + w],
                                     rhs=idn[:], start=True, stop=True)
                    nc.scalar.activation(x0sm[:w, t * P:(t + 1) * P],
                                         tp[:w, :], AFT.Copy)
                dma_slot_major_out(nc.sync, bounce[0], x0sm)
            nc.gpsimd.collective_compute(
                "AllGather", ALU.bypass, ins=[bounce[0].opt()],
                outs=[x_full[0].opt()], replica_groups=rg)

            # ================= conv layers =================
            with tc.tile_pool(name="gpool", bufs=10) as gpool, \
                 tc.tile_pool(name="ywork", bufs=3) as ywork, \
                 tc.tile_pool(name="yps", bufs=3, space="PSUM") as yps_pool, \
                 tc.tile_pool(name="tps2", bufs=4, space="PSUM") as tps2_pool:
                beta_prev = 1.0
                for l in range(NIT):
                    last = l == NIT - 1
                    beta = min(0.5, (l + 1) / cfg.NLAYERS * 0.5)
                    c1 = float((1.0 - beta) * E1)
                    nc.vector.tensor_scalar(x0sm[:], x0sm[:],
                                            float(beta / beta_prev), None,
                                            ALU.mult)
                    beta_prev = beta
                    xf = x_full[l][:]
                    halves = [xf[:HALF, :], xf[HALF:, :]]
                    gts = []
                    for (h, off, n) in calls:
                        gt = gpool.tile([P, GCAP], DT,
                                        name=f"g{l}_{off}", tag="g")
                        nc.gpsimd.dma_gather(
                            gt[:, :n].rearrange("p (n e) -> p n e", e=P),
                            halves[h],
                            idx_sb[:, off // 16:(off + n) // 16],
                            num_idxs=n, num_idxs_reg=n,
                            elem_size=P, queue_num=len(gts) % 4)
                        gts.append(gt)
                    for k in range(NPACK):
                        wk = cw(k)
                        n_mm = sum(len(sch.s_cell[(k, h)]) for h in range(2))
                        ps = yps_pool.tile([P, cfg.PACK], f32,
                                           name=f"y{l}_{k}", tag="yps")
                        nc.vector.memset(ps[:, :wk], 0.0)
                        mi = 0
                        for h in range(2):
                            base = sch.cell_off[(k, h)]
                            for ci, (co, wdt, soff) in enumerate(
                                    sch.s_cell[(k, h)]):
                                cidx, j = chunk_call[base + ci * P]
                                lv = gts[cidx][:, j * P:(j + 1) * P]
                                wdt2 = min(wdt, wk - co)
                                nc.tensor.matmul(
                                    ps[:, co:co + wdt2],
                                    lhsT=lv,
                                    rhs=s_sb[:, soff:soff + wdt2],
                                    start=False, stop=(mi == n_mm - 1),
                                    skip_group_check=True)
                                mi += 1
                        yraw = ywork.tile([P, cfg.PACK], DT,
                                          name=f"yr{l}_{k}", tag="yr")
                        nc.vector.tensor_copy(yraw[:, :wk], ps[:, :wk])
                        # transform + transpose: per 128-dest chunk
                        for c in range((wk + P - 1) // P):
                            w = min(P, wk - c * P)
                            gcol = k * cfg.PACK + c * P
                            tp = tps2_pool.tile([P, P], f32,
                                                name=f"t{l}_{k}_{c}",
                                                tag="tps2")
                            nc.tensor.matmul(
                                tp[:w, :], lhsT=yraw[:, c * P:c * P + w],
                                rhs=wh_sb[:, l * P:(l + 1) * P],
                                start=True, stop=True)
                            nc.scalar.activation(
                                a1[:w, gcol:gcol + P], tp[:w, :], AFT.Relu,
                                bias=b_relu[:w], scale=INV08)
                    # activation chain (slot-major, full width, fp16)
                    nc.vector.tensor_scalar(a1[:], a1[:], 1.0, c1,
                                            ALU.min, ALU.mult)
                    nc.scalar.activation(xact[:], a1[:], AFT.Sigmoid,
                                         scale=float(-1.0 / c1))
                    nc.vector.tensor_tensor(a1[:], a1[:], xact[:],
                                            op=ALU.mult)
                    nc.vector.tensor_tensor(xn16[:], a1[:], x0sm[:],
                                            op=ALU.add)
                    if not last:
                        dma_slot_major_out(nc.sync, bounce[l + 1], xn16)
                        nc.gpsimd.collective_compute(
                            "AllGather", ALU.bypass, ins=[bounce[l + 1].opt()],
                            outs=[x_full[l + 1].opt()], replica_groups=rg)

            # ================= output stage =================
            with tc.tile_pool(name="ow", bufs=2) as ow, \
                 tc.tile_pool(name="ops", bufs=2, space="PSUM") as ops_pool:
                for t in range(NT):
                    w = min(P, R - t * P)
                    tp = ops_pool.tile([P, P], f32, name=f"xt{t}", tag="oxt")
                    nc.tensor.matmul(
                        tp[:, :w],
                        lhsT=xn16[:w, t * P:(t + 1) * P],
                        rhs=idn[:w, :w], start=True, stop=True)
                    xnT = ow.tile([P, P], DT, name="xnT", tag="xnT")
                    nc.scalar.activation(xnT[:, :w], tp[:, :w], AFT.Copy)
                    lg = ops_pool.tile([P, cfg.NCLASS], f32,
                                       name=f"lg{t}", tag="olg")
                    nc.tensor.matmul(lg[:w, :], lhsT=xnT[:, :w],
                                     rhs=wo_sb[:], start=True, stop=True)
                    mx = ow.tile([P, 1], f32, name="mx", tag="mx")
                    nc.vector.tensor_reduce(mx[:w], lg[:w, :],
                                            axis=AX.X, op=ALU.max)
                    nmx = ow.tile([P, 1], f32, name="nmx", tag="nmx")
                    nc.vector.tensor_scalar(nmx[:w], mx[:w], -1.0,
                                            None, ALU.mult)
                    ex = ow.tile([P, cfg.NCLASS], f32, name="ex", tag="ex")
                    sume = ow.tile([P, 1], f32, name="sume", tag="sume")
                    nc.scalar.activation(ex[:w], lg[:w, :], AFT.Exp,
                                         bias=nmx[:w], accum_out=sume[:w])
                    lse = ow.tile([P, 1], f32, name="lse", tag="lse")
                    nc.scalar.activation(lse[:w], sume[:w], AFT.Ln)
                    nc.vector.tensor_tensor(lse[:w], lse[:w], nmx[:w],
                                            op=ALU.subtract)
                    res = ow.tile([P, cfg.NCLASS], f32, name="res", tag="ex")
                    nc.vector.tensor_scalar(res[:w], lg[:w, :], lse[:w],
                                            None, ALU.subtract)
                    nc.sync.dma_start(out[t * P:t * P + w, :], res[:w])
    nc.compile()
    return nc


def kernel(**inputs) -> np.ndarray:
    cfg = Cfg()
    features = np.asarray(inputs["features"], np.float32)
    edge_row = np.asarray(inputs["edge_row"], np.int64)
    edge_col = np.asarray(inputs["edge_col"], np.int64)
    W_in = np.asarray(inputs["W_in"], np.float32)
    Ws = np.asarray(inputs["Ws"], np.float32)
    c = np.asarray(inputs["c"], np.float32)
    W_out = np.asarray(inputs["W_out"], np.float32)

    in_maps, sch, dest_of = preprocess(cfg, features, edge_row, edge_col,
                                       W_in, Ws, c, W_out)
    nc = build_program(cfg, sch)

    import os
    from concourse import bass_utils
    res = bass_utils.run_bass_kernel_spmd(
        nc, in_maps, core_ids=list(range(cfg.n_cores)),
        trace=bool(os.environ.get("GNN_TRACE")))
    kernel.last_result = res
    out = np.empty((cfg.N, cfg.NCLASS), np.float32)
    for d in range(cfg.n_cores):
        out[d * cfg.R + dest_of[d]] = res.results[d]["out"]
    return out


# revision 15
# speedup vs baseline: 9.2513x; 1.0713x over previous
"""Trainium2 Bass kernel for nn_NewActivationGNN (GNN message passing).

Architecture (v2, swdge gather):
  y_l = A_norm @ (x @ W_l) is computed as (A_norm @ x) @ W_l — aggregate
  raw hidden rows first, transform after. Per-edge source rows are pulled
  straight from the DRAM-resident AllGathered node table with SWDGE
  dma_gather (one 256B descriptor per edge, slot-major destination), so
  the gather costs ~23ns/edge across 16 SDMA engines instead of ~26ns/edge
  serialized on gpsimd Q7 cores.

  Slot-major pipeline: gathered chunks [128 edge-slots, 128 hid] are lhsT
  for selection-matrix matmuls (rhs = S fp16 [slot, dest-span] carrying
  deg_inv) accumulating feature-major y_raw per 512-dest pack in PSUM;
  the W_l transform (lhsT = y_raw chunk, rhs = W) lands slot-major
  [128 dest, 128 hid] tiles; activation + residual run slot-major
  full-width; the next table is DMA'd out row-major and AllGathered.

  S matrices and the index stream are small (dest-dense cells of
  (pack, src-half)) and stay SBUF-resident across all 4 layers.

Sharding: nodes split across 8 cores by destination (graph parallel);
per-layer AllGather of the fp16 slot-major table; weights replicated.
SPMD: one program for all cores; per-device variation (indices, S
matrices, features) is data. Chunk counts / S spans are made
device-uniform by padding to the cross-device maximum.

Edge indices are int16, so the node table is addressed in two halves
(src position < 32768 vs >= 32768); cells are keyed (pack, half).
"""

import sys

for _p in ("/opt/trn_rl_repo", "/root/.axon_site/_ro/trn_rl_repo"):
    if _p not in sys.path:
        sys.path.insert(0, _p)

from dataclasses import dataclass

import numpy as np

import concourse.bass as bass  # noqa: F401
import concourse.tile as tile
from concourse import bacc, mybir
from concourse.masks import make_identity

P = 128
HALF = 32768  # int16 index range per table half


@dataclass
class Cfg:
    N: int = 50000
    E: int = 800000
    NFEAT: int = 500
    NHID: int = 128
    NCLASS: int = 40
    NLAYERS: int = 4
    GAMMA: float = 0.3
    X1: float = 0.1
    X2: float = 0.9
    C_ACT: float = -1.0
    n_cores: int = 8
    PACK: int = 512

    @property
    def R(self):
        return self.N // self.n_cores          # 6250 dest rows per core

    @property
    def NPACK(self):
        return (self.R + self.PACK - 1) // self.PACK   # 13

    @property
    def NFP(self):
        return ((self.NFEAT + 1 + P - 1) // P) * P     # 512


class Sched:
    """Device-uniform schedule over cells keyed (pack k, half h)."""

    def __init__(self, cfg, counts, spans):
        self.B = counts.max(axis=0)                      # [NPACK, 2]
        self.nch = (self.B + P - 1) // P
        self.cell_off = {}
        off = 0
        for k in range(cfg.NPACK):
            for h in range(2):
                self.cell_off[(k, h)] = off
                off += int(self.nch[k, h]) * P
        self.idx_total = off
        s_off = 0
        self.s_cell = {}
        for k in range(cfg.NPACK):
            for h in range(2):
                metas = []
                for ci in range(int(self.nch[k, h])):
                    co, w = spans[(k, h, ci)]
                    metas.append((co, w, s_off))
                    s_off += w
                self.s_cell[(k, h)] = metas
        self.s_total = s_off


def preprocess(cfg: Cfg, features, edge_row, edge_col, W_in, Ws, c, W_out):
    N, R, ncores = cfg.N, cfg.R, cfg.n_cores
    NPACK, PACK = cfg.NPACK, cfg.PACK
    f32 = np.float32

    deg = np.bincount(edge_row, minlength=N)
    deg_inv = (1.0 / np.maximum(deg, 1)).astype(f32)
    owner = edge_row // R

    # Node permutation: within each device, order nodes by descending global
    # degree so cumulative degree profiles align across devices (smaller S
    # spans, less cell padding). pos[] maps orig node id -> its table
    # position within the owning device's slice.
    dest_of = np.empty((ncores, R), np.int64)   # sorted pos -> orig local
    pos = np.empty(N, np.int64)
    for d in range(ncores):
        order = np.argsort(-deg[d * R:(d + 1) * R], kind="stable")
        dest_of[d] = order
        local_of = np.empty(R, np.int64)
        local_of[order] = np.arange(R)
        pos[d * R:(d + 1) * R] = local_of

    dev = []
    counts = np.zeros((ncores, NPACK, 2), np.int64)
    for d in range(ncores):
        m = owner == d
        dl = pos[edge_row[m]]
        src = edge_col[m].astype(np.int64)
        gp = (src // R) * R + pos[src]          # global table position
        h = gp // HALF                          # table half (0 or 1)
        iv = gp - h * HALF                      # int16-safe row index
        k = dl // PACK
        so = np.lexsort((dl, h, k))
        dl, h, iv, k = dl[so], h[so], iv[so], k[so]
        cell_id = k * 2 + h
        cnt = np.bincount(cell_id, minlength=NPACK * 2)
        counts[d] = cnt.reshape(NPACK, 2)
        dev.append((dl, iv, cell_id))

    B = counts.max(axis=0)
    nch = (B + P - 1) // P

    spans = {}
    dev_cell_start = []
    for d in range(ncores):
        cnt = counts[d].reshape(-1)
        dev_cell_start.append(np.concatenate([[0], np.cumsum(cnt)]))
    for k in range(NPACK):
        for h in range(2):
            cid = k * 2 + h
            for ci in range(int(nch[k, h])):
                fd, ld = [], []
                for d in range(ncores):
                    dl = dev[d][0]
                    b = dev_cell_start[d][cid]
                    n_d = counts[d, k, h]
                    s0 = ci * P
                    if s0 < n_d:
                        s1 = min(s0 + P, n_d)
                        fd.append(int(dl[b + s0]))
                        ld.append(int(dl[b + s1 - 1]))
                co = min(fd) - k * PACK
                w = max(ld) - min(fd) + 1
                spans[(k, h, ci)] = (co, w)

    sch = Sched(cfg, counts, spans)

    # shared weights
    NFP = cfg.NFP
    W_aug = np.zeros((NFP, cfg.NHID), f32)
    W_aug[:cfg.NFEAT] = (1.0 - cfg.GAMMA) * W_in
    W_aug[cfg.NFEAT] = cfg.GAMMA * np.maximum(c, 0.0)
    nk = NFP // P
    W_dram = np.empty((P, nk * P), np.float16)
    for kk in range(nk):
        W_dram[:, kk * P:(kk + 1) * P] = W_aug[kk * P:(kk + 1) * P]
    Ws_dram = np.empty((P, cfg.NLAYERS * P), np.float16)
    for l in range(cfg.NLAYERS):
        Ws_dram[:, l * P:(l + 1) * P] = Ws[l]
    Wout_dram = np.ascontiguousarray(W_out).astype(np.float16)

    in_maps = []
    for d in range(ncores):
        dl, iv, cell_id = dev[d]
        cstart = dev_cell_start[d]
        idx_vals = np.zeros(sch.idx_total, np.int16)
        s_data = np.zeros((P, sch.s_total), np.float16)
        for k in range(NPACK):
            for h in range(2):
                cid = k * 2 + h
                n_d = int(counts[d, k, h])
                if n_d == 0:
                    continue
                b = cstart[cid]
                o = sch.cell_off[(k, h)]
                idx_vals[o:o + n_d] = iv[b:b + n_d].astype(np.int16)
                metas = sch.s_cell[(k, h)]
                sl = np.arange(n_d)
                ci_arr = sl // P
                row = sl % P
                co = np.array([m[0] for m in metas], np.int64)[ci_arr]
                soff = np.array([m[2] for m in metas], np.int64)[ci_arr]
                col = soff + (dl[b:b + n_d] - k * PACK - co)
                s_data[row, col] = deg_inv[
                    d * R + dest_of[d][dl[b:b + n_d]]]
        idx_t = np.tile(idx_vals.reshape(-1, 16).T, (8, 1))

        gids = d * R + dest_of[d]
        featT = np.zeros((NFP, R), np.float16)
        featT[:cfg.NFEAT] = features[gids].T
        featT[cfg.NFEAT] = 1.0

        in_maps.append(dict(
            featT=featT, idx_all=np.ascontiguousarray(idx_t), s_all=s_data,
            w_proj=W_dram, w_hid=Ws_dram, w_out=Wout_dram,
        ))
    return in_maps, sch, dest_of


def build_program(cfg: Cfg, sch: Sched):
    nc = bacc.Bacc("TRN2", target_bir_lowering=False, debug=False,
                   num_devices=cfg.n_cores, num_swdge_queues=4)
    DT = mybir.dt.float16
    f32 = mybir.dt.float32
    R, NPACK, NFP = cfg.R, cfg.NPACK, cfg.NFP
    AFT = mybir.ActivationFunctionType
    ALU = mybir.AluOpType
    AX = mybir.AxisListType
    RED = bass.bass_isa.ReduceOp
    rg = [list(range(cfg.n_cores))]
    nk = NFP // P
    NCHK = (R + cfg.PACK - 1) // cfg.PACK   # 512-col chunks over R
    NT = R // P + (1 if R % P else 0)       # 128-col tiles over R (49)

    def cw(ch):
        return min(cfg.PACK, R - ch * cfg.PACK)

    featT = nc.dram_tensor("featT", [NFP, R], DT, kind="ExternalInput").ap()
    idx_all = nc.dram_tensor("idx_all", [P, sch.idx_total // 16],
                             mybir.dt.int16, kind="ExternalInput").ap()
    s_all = nc.dram_tensor("s_all", [P, sch.s_total], DT,
                           kind="ExternalInput").ap()
    w_proj = nc.dram_tensor("w_proj", [P, nk * P], DT,
                            kind="ExternalInput").ap()
    w_hid = nc.dram_tensor("w_hid", [P, cfg.NLAYERS * P], DT,
                           kind="ExternalInput").ap()
    w_out = nc.dram_tensor("w_out", [P, cfg.NCLASS], DT,
                           kind="ExternalInput").ap()
    out = nc.dram_tensor("out", [R, cfg.NCLASS], f32,
                         kind="ExternalOutput").ap()

    INV08 = float(np.float32(1.0 / (np.float64(cfg.X2) - cfg.X1 + 1e-8)))
    B_RELU = float(np.float32(-cfg.X1 * INV08))
    E1 = float(1.0 + np.exp(-cfg.C_ACT))
    NIT = cfg.NLAYERS
    FT = R // P          # full 128-row tiles (48)
    TAIL = R - FT * P    # ragged tail rows (106)
    SMW = NT * P         # slot-major width (6272)

    # gather calls: group cells into calls of <= GCAP slots (contiguous in
    # the idx stream). Each call is one dma_gather against one table half.
    # GCAP is bounded by the SWDGE descriptor-ring carveout: one call's
    # descriptors (num_idxs/16 + 1 per DMA engine) must fit the ring or the
    # Q7 waits forever (hardware-verified: 1024 ok, 2560 wedges).
    GCAP = 1024
    calls = []     # (half, slot_off, n_slots)
    for k in range(NPACK):
        for h in range(2):
            n = int(sch.nch[k, h]) * P
            off = sch.cell_off[(k, h)]
            while n > 0:
                take = min(n, GCAP)
                # merge with previous call if same half and contiguous
                if calls and calls[-1][0] == h and \
                        calls[-1][1] + calls[-1][2] == off and \
                        calls[-1][2] + take <= GCAP:
                    calls[-1] = (h, calls[-1][1], calls[-1][2] + take)
                else:
                    calls.append((h, off, take))
                off += take
                n -= take
    # map chunk -> (call index, chunk offset within call)
    chunk_call = {}
    for idx_c, (h, off, n) in enumerate(calls):
        for j in range(n // P):
            chunk_call[off + j * P] = (idx_c, j)

    with tile.TileContext(nc) as tc:
        with tc.tile_pool(name="persist", bufs=1) as persist, \
             tc.tile_pool(name="dram", bufs=1, space="DRAM") as dram:
            idx_sb = persist.tile([P, sch.idx_total // 16], mybir.dt.int16)
            nc.sync.dma_start(idx_sb[:], idx_all[:])
            s_sb = persist.tile([P, sch.s_total], DT)
            nc.scalar.dma_start(s_sb[:], s_all[:])
            wh_sb = persist.tile([P, cfg.NLAYERS * P], DT)
            nc.sync.dma_start(wh_sb[:], w_hid[:])
            wo_sb = persist.tile([P, cfg.NCLASS], DT)
            nc.sync.dma_start(wo_sb[:], w_out[:])
            wp_sb = persist.tile([P, nk * P], DT)
            nc.sync.dma_start(wp_sb[:], w_proj[:])
            ones1 = persist.tile([1, P], f32)
            nc.vector.memset(ones1[:], 1.0)
            b_relu = persist.tile([P, 1], f32)
            nc.vector.memset(b_relu[:], B_RELU)
            idn = persist.tile([P, P], DT)
            make_identity(nc, idn[:])
            mm_sb = persist.tile([P, 2], f32)
            mm_ar = persist.tile([P, 2], f32)
            mm_back = persist.tile([1, 2], f32)
            sfac = persist.tile([P, 1], f32)
            bfac = persist.tile([P, 1], f32)
            x0sm = persist.tile([P, SMW], DT)   # slot-major x0 (beta-scaled)
            a1 = persist.tile([P, SMW], DT)     # slot-major activation buf
            # tail-tile pad lanes (partitions >= TAIL of the last 128-col
            # block) are never written by the tiled producers but are read
            # by full-width elementwise ops — zero them once.
            nc.vector.memset(x0sm[:, FT * P:], 0.0)
            nc.vector.memset(a1[:, FT * P:], 0.0)
            xact = persist.tile([P, SMW], DT)   # sigmoid scratch
            xn16 = persist.tile([P, SMW], DT)   # slot-major layer output

            def dma_slot_major_out(eng, bnc, src):
                # src slot-major [P, SMW] -> bnc row-major [R, P]
                eng.dma_start(
                    bnc[:FT * P, :].rearrange("(t p) h -> p t h", p=P),
                    src[:, :FT * P].rearrange("p (t h) -> p t h", h=P))
                eng.dma_start(bnc[FT * P:R, :], src[:TAIL, FT * P:FT * P + P])

            bounce = [dram.tile([R, P], DT, name=f"bounce{i}")
                      for i in range(NIT)]
            x_full = [dram.tile([cfg.N, P], DT, addr_space="Shared",
                                name=f"x_full{i}") for i in range(NIT)]
            mm_in = dram.tile([1, 2], f32)
            mm_out = dram.tile([1, 2], f32, addr_space="Shared")

            # ================= projection =================
            with tc.tile_pool(name="strips", bufs=1) as strip_pool, \
                 tc.tile_pool(name="pwork", bufs=2) as pwork, \
                 tc.tile_pool(name="pps", bufs=2, space="PSUM") as pps_pool, \
                 tc.tile_pool(name="tps", bufs=4, space="PSUM") as tps_pool:
                strips = []
                for k in range(nk):
                    st = strip_pool.tile([P, R], DT, name=f"strip{k}",
                                         tag=f"strip{k}")
                    nc.sync.dma_start(st[:], featT[k * P:(k + 1) * P, :])
                    strips.append(st)
                x0T = strip_pool.tile([P, R], DT, name="x0T", tag="x0T")
                yT = strip_pool.tile([P, R], f32, name="yT", tag="yT")
                rmax = pwork.tile([P, 1], f32, name="rmax", tag="rmax")
                rmin = pwork.tile([P, 1], f32, name="rmin", tag="rmin")
                for ch in range(NCHK):
                    w = cw(ch)
                    sl = slice(ch * cfg.PACK, ch * cfg.PACK + w)
                    ps = pps_pool.tile([P, cfg.PACK], f32, name=f"h{ch}",
                                       tag="hps")
                    for k in range(nk):
                        nc.tensor.matmul(ps[:, :w],
                                         lhsT=wp_sb[:, k * P:(k + 1) * P],
                                         rhs=strips[k][:, sl],
                                         start=(k == 0), stop=(k == nk - 1))
                    nc.vector.tensor_copy(yT[:, sl], ps[:, :w])
                    qmax = pwork.tile([P, 1], f32, name="qmax", tag="qmax")
                    qmin = pwork.tile([P, 1], f32, name="qmin", tag="qmin")
                    nc.vector.tensor_reduce(qmax[:], ps[:, :w], axis=AX.X,
                                            op=ALU.max)
                    nc.vector.tensor_reduce(qmin[:], ps[:, :w], axis=AX.X,
                                            op=ALU.min)
                    if ch == 0:
                        nc.vector.tensor_copy(rmax[:], qmax[:])
                        nc.vector.tensor_copy(rmin[:], qmin[:])
                    else:
                        nc.vector.tensor_tensor(rmax[:], rmax[:], qmax[:],
                                                op=ALU.max)
                        nc.vector.tensor_tensor(rmin[:], rmin[:], qmin[:],
                                                op=ALU.min)
                nc.vector.tensor_copy(mm_sb[:, 0:1], rmax[:])
                nc.vector.tensor_scalar(mm_sb[:, 1:2], rmin[:], -1.0, None,
                                        ALU.mult)
                nc.gpsimd.partition_all_reduce(mm_ar[:], mm_sb[:],
                                               channels=P, reduce_op=RED.max)
                nc.sync.dma_start(mm_in[:], mm_ar[0:1, :])
                nc.gpsimd.collective_compute(
                    "AllReduce", ALU.max, ins=[mm_in.opt()],
                    outs=[mm_out.opt()], replica_groups=rg)
                nc.sync.dma_start(mm_back[:], mm_out[:])
                bc_ps = pps_pool.tile([P, 2], f32, name="bc_ps", tag="hps")
                nc.tensor.matmul(bc_ps[:], lhsT=ones1[:], rhs=mm_back[:],
                                 start=True, stop=True)
                bcast = pwork.tile([P, 2], f32, name="bcast", tag="qmin")
                nc.vector.tensor_copy(bcast[:], bc_ps[:])
                sden = pwork.tile([P, 1], f32, name="sden", tag="qmax")
                nc.vector.tensor_tensor(sden[:], bcast[:, 0:1], bcast[:, 1:2],
                                        op=ALU.add)
                nc.vector.tensor_scalar(sden[:], sden[:], 1e-8, None, ALU.add)
                nc.vector.reciprocal(sfac[:], sden[:])
                nc.vector.tensor_tensor(bfac[:], bcast[:, 1:2], sfac[:],
                                        op=ALU.mult)
                nc.vector.tensor_scalar(x0T[:], yT[:], sfac[:], bfac[:],
                                        ALU.mult, ALU.add)
                # transpose x0T (feature-major) -> x0sm (slot-major)
                for t in range(NT):
                    w = min(P, R - t * P)
                    tp = tps_pool.tile([P, P], f32, name=f"x0t{t}", tag="tps")
                    nc.tensor.matmul(tp[:w, :], lhsT=x0T[:, t * P:t * P + w],
                                     rhs=idn[:], start=True, stop=True)
                    nc.scalar.activation(x0sm[:w, t * P:(t + 1) * P],
                                         tp[:w, :], AFT.Copy)
                dma_slot_major_out(nc.sync, bounce[0], x0sm)
            nc.gpsimd.collective_compute(
                "AllGather", ALU.bypass, ins=[bounce[0].opt()],
                outs=[x_full[0].opt()], replica_groups=rg)

            # ================= conv layers =================
            with tc.tile_pool(name="gpool", bufs=16) as gpool, \
                 tc.tile_pool(name="ywork", bufs=3) as ywork, \
                 tc.tile_pool(name="yps", bufs=3, space="PSUM") as yps_pool, \
                 tc.tile_pool(name="tps2", bufs=4, space="PSUM") as tps2_pool:
                beta_prev = 1.0
                for l in range(NIT):
                    last = l == NIT - 1
                    beta = min(0.5, (l + 1) / cfg.NLAYERS * 0.5)
                    c1 = float((1.0 - beta) * E1)
                    nc.vector.tensor_scalar(x0sm[:], x0sm[:],
                                            float(beta / beta_prev), None,
                                            ALU.mult)
                    beta_prev = beta
                    xf = x_full[l][:]
                    halves = [xf[:HALF, :], xf[HALF:, :]]
                    gts = []
                    for (h, off, n) in calls:
                        gt = gpool.tile([P, GCAP], DT,
                                        name=f"g{l}_{off}", tag="g")
                        nc.gpsimd.dma_gather(
                            gt[:, :n].rearrange("p (n e) -> p n e", e=P),
                            halves[h],
                            idx_sb[:, off // 16:(off + n) // 16],
                            num_idxs=n, num_idxs_reg=n,
                            elem_size=P, queue_num=len(gts) % 4)
                        gts.append(gt)
                    for k in range(NPACK):
                        wk = cw(k)
                        n_mm = sum(len(sch.s_cell[(k, h)]) for h in range(2))
                        ps = yps_pool.tile([P, cfg.PACK], f32,
                                           name=f"y{l}_{k}", tag="yps")
                        nc.vector.memset(ps[:, :wk], 0.0)
                        mi = 0
                        for h in range(2):
                            base = sch.cell_off[(k, h)]
                            for ci, (co, wdt, soff) in enumerate(
                                    sch.s_cell[(k, h)]):
                                cidx, j = chunk_call[base + ci * P]
                                lv = gts[cidx][:, j * P:(j + 1) * P]
                                wdt2 = min(wdt, wk - co)
                                nc.tensor.matmul(
                                    ps[:, co:co + wdt2],
                                    lhsT=lv,
                                    rhs=s_sb[:, soff:soff + wdt2],
                                    start=False, stop=(mi == n_mm - 1),
                                    skip_group_check=True)
                                mi += 1
                        yraw = ywork.tile([P, cfg.PACK], DT,
                                          name=f"yr{l}_{k}", tag="yr")
                        nc.vector.tensor_copy(yraw[:, :wk], ps[:, :wk])
                        # transform + transpose: per 128-dest chunk
                        for c in range((wk + P - 1) // P):
                            w = min(P, wk - c * P)
                            gcol = k * cfg.PACK + c * P
                            tp = tps2_pool.tile([P, P], f32,
                                                name=f"t{l}_{k}_{c}",
                                                tag="tps2")
                            nc.tensor.matmul(
                                tp[:w, :], lhsT=yraw[:, c * P:c * P + w],
                                rhs=wh_sb[:, l * P:(l + 1) * P],
                                start=True, stop=True)
                            nc.scalar.activation(
                                a1[:w, gcol:gcol + P], tp[:w, :], AFT.Relu,
                                bias=b_relu[:w], scale=INV08)
                    # activation chain (slot-major, full width, fp16)
                    nc.vector.tensor_scalar(a1[:], a1[:], 1.0, c1,
                                            ALU.min, ALU.mult)
                    nc.scalar.activation(xact[:], a1[:], AFT.Sigmoid,
                                         scale=float(-1.0 / c1))
                    nc.vector.tensor_tensor(a1[:], a1[:], xact[:],
                                            op=ALU.mult)
                    nc.vector.tensor_tensor(xn16[:], a1[:], x0sm[:],
                                            op=ALU.add)
                    if not last:
                        dma_slot_major_out(nc.sync, bounce[l + 1], xn16)
                        nc.gpsimd.collective_compute(
                            "AllGather", ALU.bypass, ins=[bounce[l + 1].opt()],
                            outs=[x_full[l + 1].opt()], replica_groups=rg)

            # ================= output stage =================
            with tc.tile_pool(name="ow", bufs=2) as ow, \
                 tc.tile_pool(name="ops", bufs=2, space="PSUM") as ops_pool:
                for t in range(NT):
                    w = min(P, R - t * P)
                    tp = ops_pool.tile([P, P], f32, name=f"xt{t}", tag="oxt")
                    nc.tensor.matmul(
                        tp[:, :w],
                        lhsT=xn16[:w, t * P:(t + 1) * P],
                        rhs=idn[:w, :w], start=True, stop=True)
                    xnT = ow.tile([P, P], DT, name="xnT", tag="xnT")
                    nc.scalar.activation(xnT[:, :w], tp[:, :w], AFT.Copy)
                    lg = ops_pool.tile([P, cfg.NCLASS], f32,
                                       name=f"lg{t}", tag="olg")
                    nc.tensor.matmul(lg[:w, :], lhsT=xnT[:, :w],
                                     rhs=wo_sb[:], start=True, stop=True)
                    mx = ow.tile([P, 1], f32, name="mx", tag="mx")
                    nc.vector.tensor_reduce(mx[:w], lg[:w, :],
                                            axis=AX.X, op=ALU.max)
                    nmx = ow.tile([P, 1], f32, name="nmx", tag="nmx")
                    nc.vector.tensor_scalar(nmx[:w], mx[:w], -1.0,
                                            None, ALU.mult)
                    ex = ow.tile([P, cfg.NCLASS], f32, name="ex", tag="ex")
                    sume = ow.tile([P, 1], f32, name="sume", tag="sume")
                    nc.scalar.activation(ex[:w], lg[:w, :], AFT.Exp,
                                         bias=nmx[:w], accum_out=sume[:w])
                    lse = ow.tile([P, 1], f32, name="lse", tag="lse")
                    nc.scalar.activation(lse[:w], sume[:w], AFT.Ln)
                    nc.vector.tensor_tensor(lse[:w], lse[:w], nmx[:w],
                                            op=ALU.subtract)
                    res = ow.tile([P, cfg.NCLASS], f32, name="res", tag="ex")
                    nc.vector.tensor_scalar(res[:w], lg[:w, :], lse[:w],
                                            None, ALU.subtract)
                    nc.sync.dma_start(out[t * P:t * P + w, :], res[:w])
    nc.compile()
    return nc


def kernel(**inputs) -> np.ndarray:
    cfg = Cfg()
    features = np.asarray(inputs["features"], np.float32)
    edge_row = np.asarray(inputs["edge_row"], np.int64)
    edge_col = np.asarray(inputs["edge_col"], np.int64)
    W_in = np.asarray(inputs["W_in"], np.float32)
    Ws = np.asarray(inputs["Ws"], np.float32)
    c = np.asarray(inputs["c"], np.float32)
    W_out = np.asarray(inputs["W_out"], np.float32)

    in_maps, sch, dest_of = preprocess(cfg, features, edge_row, edge_col,
                                       W_in, Ws, c, W_out)
    nc = build_program(cfg, sch)

    import os
    from concourse import bass_utils
    res = bass_utils.run_bass_kernel_spmd(
        nc, in_maps, core_ids=list(range(cfg.n_cores)),
        trace=bool(os.environ.get("GNN_TRACE")))
    kernel.last_result = res
    out = np.empty((cfg.N, cfg.NCLASS), np.float32)
    for d in range(cfg.n_cores):
        out[d * cfg.R + dest_of[d]] = res.results[d]["out"]
    return out


# revision 17
# speedup vs baseline: 9.6589x; 1.0441x over previous
"""Trainium2 Bass kernel for nn_NewActivationGNN (GNN message passing).

Architecture (v2, swdge gather):
  y_l = A_norm @ (x @ W_l) is computed as (A_norm @ x) @ W_l — aggregate
  raw hidden rows first, transform after. Per-edge source rows are pulled
  straight from the DRAM-resident AllGathered node table with SWDGE
  dma_gather (one 256B descriptor per edge, slot-major destination), so
  the gather costs ~23ns/edge across 16 SDMA engines instead of ~26ns/edge
  serialized on gpsimd Q7 cores.

  Slot-major pipeline: gathered chunks [128 edge-slots, 128 hid] are lhsT
  for selection-matrix matmuls (rhs = S fp16 [slot, dest-span] carrying
  deg_inv) accumulating feature-major y_raw per 512-dest pack in PSUM;
  the W_l transform (lhsT = y_raw chunk, rhs = W) lands slot-major
  [128 dest, 128 hid] tiles; activation + residual run slot-major
  full-width; the next table is DMA'd out row-major and AllGathered.

  S matrices and the index stream are small (dest-dense cells of
  (pack, src-half)) and stay SBUF-resident across all 4 layers.

Sharding: nodes split across 8 cores by destination (graph parallel);
per-layer AllGather of the fp16 slot-major table; weights replicated.
SPMD: one program for all cores; per-device variation (indices, S
matrices, features) is data. Chunk counts / S spans are made
device-uniform by padding to the cross-device maximum.

Edge indices are int16, so the node table is addressed in two halves
(src position < 32768 vs >= 32768); cells are keyed (pack, half).
"""

import sys

for _p in ("/opt/trn_rl_repo", "/root/.axon_site/_ro/trn_rl_repo"):
    if _p not in sys.path:
        sys.path.insert(0, _p)

from dataclasses import dataclass

import numpy as np

import concourse.bass as bass  # noqa: F401
import concourse.tile as tile
from concourse import bacc, mybir
from concourse.masks import make_identity

P = 128
HALF = 32768  # int16 index range per table half


@dataclass
class Cfg:
    N: int = 50000
    E: int = 800000
    NFEAT: int = 500
    NHID: int = 128
    NCLASS: int = 40
    NLAYERS: int = 4
    GAMMA: float = 0.3
    X1: float = 0.1
    X2: float = 0.9
    C_ACT: float = -1.0
    n_cores: int = 8
    PACK: int = 512

    @property
    def R(self):
        return self.N // self.n_cores          # 6250 dest rows per core

    @property
    def NPACK(self):
        return (self.R + self.PACK - 1) // self.PACK   # 13

    @property
    def NFP(self):
        return ((self.NFEAT + 1 + P - 1) // P) * P     # 512


class Sched:
    """Device-uniform schedule over cells keyed (pack k, half h)."""

    def __init__(self, cfg, counts, spans):
        self.B = counts.max(axis=0)                      # [NPACK, 2]
        self.nch = (self.B + P - 1) // P
        self.cell_off = {}
        off = 0
        for k in range(cfg.NPACK):
            for h in range(2):
                self.cell_off[(k, h)] = off
                off += int(self.nch[k, h]) * P
        self.idx_total = off
        s_off = 0
        self.s_cell = {}
        for k in range(cfg.NPACK):
            for h in range(2):
                metas = []
                for ci in range(int(self.nch[k, h])):
                    co, w = spans[(k, h, ci)]
                    metas.append((co, w, s_off))
                    s_off += w
                self.s_cell[(k, h)] = metas
        self.s_total = s_off


def preprocess(cfg: Cfg, features, edge_row, edge_col, W_in, Ws, c, W_out):
    N, R, ncores = cfg.N, cfg.R, cfg.n_cores
    NPACK, PACK = cfg.NPACK, cfg.PACK
    f32 = np.float32

    deg = np.bincount(edge_row, minlength=N)
    deg_inv = (1.0 / np.maximum(deg, 1)).astype(f32)
    owner = edge_row // R

    # Node permutation: within each device, order nodes by descending global
    # degree so cumulative degree profiles align across devices (smaller S
    # spans, less cell padding). pos[] maps orig node id -> its table
    # position within the owning device's slice.
    dest_of = np.empty((ncores, R), np.int64)   # sorted pos -> orig local
    pos = np.empty(N, np.int64)
    for d in range(ncores):
        order = np.argsort(-deg[d * R:(d + 1) * R], kind="stable")
        dest_of[d] = order
        local_of = np.empty(R, np.int64)
        local_of[order] = np.arange(R)
        pos[d * R:(d + 1) * R] = local_of

    dev = []
    counts = np.zeros((ncores, NPACK, 2), np.int64)
    for d in range(ncores):
        m = owner == d
        dl = pos[edge_row[m]]
        src = edge_col[m].astype(np.int64)
        gp = (src // R) * R + pos[src]          # global table position
        h = gp // HALF                          # table half (0 or 1)
        iv = gp - h * HALF                      # int16-safe row index
        k = dl // PACK
        so = np.lexsort((dl, h, k))
        dl, h, iv, k = dl[so], h[so], iv[so], k[so]
        cell_id = k * 2 + h
        cnt = np.bincount(cell_id, minlength=NPACK * 2)
        counts[d] = cnt.reshape(NPACK, 2)
        dev.append((dl, iv, cell_id))

    B = counts.max(axis=0)
    nch = (B + P - 1) // P

    spans = {}
    dev_cell_start = []
    for d in range(ncores):
        cnt = counts[d].reshape(-1)
        dev_cell_start.append(np.concatenate([[0], np.cumsum(cnt)]))
    for k in range(NPACK):
        for h in range(2):
            cid = k * 2 + h
            for ci in range(int(nch[k, h])):
                fd, ld = [], []
                for d in range(ncores):
                    dl = dev[d][0]
                    b = dev_cell_start[d][cid]
                    n_d = counts[d, k, h]
                    s0 = ci * P
                    if s0 < n_d:
                        s1 = min(s0 + P, n_d)
                        fd.append(int(dl[b + s0]))
                        ld.append(int(dl[b + s1 - 1]))
                co = min(fd) - k * PACK
                w = max(ld) - min(fd) + 1
                spans[(k, h, ci)] = (co, w)

    sch = Sched(cfg, counts, spans)

    # shared weights
    NFP = cfg.NFP
    W_aug = np.zeros((NFP, cfg.NHID), f32)
    W_aug[:cfg.NFEAT] = (1.0 - cfg.GAMMA) * W_in
    W_aug[cfg.NFEAT] = cfg.GAMMA * np.maximum(c, 0.0)
    nk = NFP // P
    W_dram = np.empty((P, nk * P), np.float16)
    for kk in range(nk):
        W_dram[:, kk * P:(kk + 1) * P] = W_aug[kk * P:(kk + 1) * P]
    Ws_dram = np.empty((P, cfg.NLAYERS * P), np.float16)
    for l in range(cfg.NLAYERS):
        Ws_dram[:, l * P:(l + 1) * P] = Ws[l]
    Wout_dram = np.ascontiguousarray(W_out).astype(np.float16)

    in_maps = []
    for d in range(ncores):
        dl, iv, cell_id = dev[d]
        cstart = dev_cell_start[d]
        idx_vals = np.zeros(sch.idx_total, np.int16)
        s_data = np.zeros((P, sch.s_total), np.float16)
        for k in range(NPACK):
            for h in range(2):
                cid = k * 2 + h
                n_d = int(counts[d, k, h])
                if n_d == 0:
                    continue
                b = cstart[cid]
                o = sch.cell_off[(k, h)]
                idx_vals[o:o + n_d] = iv[b:b + n_d].astype(np.int16)
                metas = sch.s_cell[(k, h)]
                sl = np.arange(n_d)
                ci_arr = sl // P
                row = sl % P
                co = np.array([m[0] for m in metas], np.int64)[ci_arr]
                soff = np.array([m[2] for m in metas], np.int64)[ci_arr]
                col = soff + (dl[b:b + n_d] - k * PACK - co)
                s_data[row, col] = deg_inv[
                    d * R + dest_of[d][dl[b:b + n_d]]]
        idx_t = np.tile(idx_vals.reshape(-1, 16).T, (8, 1))

        gids = d * R + dest_of[d]
        featT = np.zeros((NFP, R), np.float16)
        featT[:cfg.NFEAT] = features[gids].T
        featT[cfg.NFEAT] = 1.0

        in_maps.append(dict(
            featT=featT, idx_all=np.ascontiguousarray(idx_t), s_all=s_data,
            w_proj=W_dram, w_hid=Ws_dram, w_out=Wout_dram,
        ))
    return in_maps, sch, dest_of


def build_program(cfg: Cfg, sch: Sched):
    nc = bacc.Bacc("TRN2", target_bir_lowering=False, debug=False,
                   num_devices=cfg.n_cores, num_swdge_queues=4)
    DT = mybir.dt.float16
    f32 = mybir.dt.float32
    R, NPACK, NFP = cfg.R, cfg.NPACK, cfg.NFP
    AFT = mybir.ActivationFunctionType
    ALU = mybir.AluOpType
    AX = mybir.AxisListType
    RED = bass.bass_isa.ReduceOp
    rg = [list(range(cfg.n_cores))]
    nk = NFP // P
    NCHK = (R + cfg.PACK - 1) // cfg.PACK   # 512-col chunks over R
    NT = R // P + (1 if R % P else 0)       # 128-col tiles over R (49)

    def cw(ch):
        return min(cfg.PACK, R - ch * cfg.PACK)

    featT = nc.dram_tensor("featT", [NFP, R], DT, kind="ExternalInput").ap()
    idx_all = nc.dram_tensor("idx_all", [P, sch.idx_total // 16],
                             mybir.dt.int16, kind="ExternalInput").ap()
    s_all = nc.dram_tensor("s_all", [P, sch.s_total], DT,
                           kind="ExternalInput").ap()
    w_proj = nc.dram_tensor("w_proj", [P, nk * P], DT,
                            kind="ExternalInput").ap()
    w_hid = nc.dram_tensor("w_hid", [P, cfg.NLAYERS * P], DT,
                           kind="ExternalInput").ap()
    w_out = nc.dram_tensor("w_out", [P, cfg.NCLASS], DT,
                           kind="ExternalInput").ap()
    out = nc.dram_tensor("out", [R, cfg.NCLASS], f32,
                         kind="ExternalOutput").ap()

    INV08 = float(np.float32(1.0 / (np.float64(cfg.X2) - cfg.X1 + 1e-8)))
    B_RELU = float(np.float32(-cfg.X1 * INV08))
    E1 = float(1.0 + np.exp(-cfg.C_ACT))
    NIT = cfg.NLAYERS
    FT = R // P          # full 128-row tiles (48)
    TAIL = R - FT * P    # ragged tail rows (106)
    SMW = NT * P         # slot-major width (6272)

    # gather calls: group cells into calls of <= GCAP slots (contiguous in
    # the idx stream). Each call is one dma_gather against one table half.
    # GCAP is bounded by the SWDGE descriptor-ring carveout: one call's
    # descriptors (num_idxs/16 + 1 per DMA engine) must fit the ring or the
    # Q7 waits forever (hardware-verified: 1024 ok, 2560 wedges).
    GCAP = 1024
    calls = []     # (half, slot_off, n_slots)
    for k in range(NPACK):
        for h in range(2):
            n = int(sch.nch[k, h]) * P
            off = sch.cell_off[(k, h)]
            while n > 0:
                take = min(n, GCAP)
                # merge with previous call if same half and contiguous
                if calls and calls[-1][0] == h and \
                        calls[-1][1] + calls[-1][2] == off and \
                        calls[-1][2] + take <= GCAP:
                    calls[-1] = (h, calls[-1][1], calls[-1][2] + take)
                else:
                    calls.append((h, off, take))
                off += take
                n -= take
    # map chunk -> (call index, chunk offset within call)
    chunk_call = {}
    for idx_c, (h, off, n) in enumerate(calls):
        for j in range(n // P):
            chunk_call[off + j * P] = (idx_c, j)

    with tile.TileContext(nc) as tc:
        with tc.tile_pool(name="persist", bufs=1) as persist, \
             tc.tile_pool(name="dram", bufs=1, space="DRAM") as dram:
            idx_sb = persist.tile([P, sch.idx_total // 16], mybir.dt.int16)
            nc.sync.dma_start(idx_sb[:], idx_all[:])
            s_sb = persist.tile([P, sch.s_total], DT)
            nc.scalar.dma_start(s_sb[:], s_all[:])
            wh_sb = persist.tile([P, cfg.NLAYERS * P], DT)
            nc.sync.dma_start(wh_sb[:], w_hid[:])
            wo_sb = persist.tile([P, cfg.NCLASS], DT)
            nc.sync.dma_start(wo_sb[:], w_out[:])
            wp_sb = persist.tile([P, nk * P], DT)
            nc.sync.dma_start(wp_sb[:], w_proj[:])
            ones1 = persist.tile([1, P], f32)
            nc.vector.memset(ones1[:], 1.0)
            b_relu = persist.tile([P, 1], f32)
            nc.vector.memset(b_relu[:], B_RELU)
            idn = persist.tile([P, P], DT)
            make_identity(nc, idn[:])
            mm_sb = persist.tile([P, 2], f32)
            mm_ar = persist.tile([P, 2], f32)
            mm_back = persist.tile([1, 2], f32)
            sfac = persist.tile([P, 1], f32)
            bfac = persist.tile([P, 1], f32)
            x0sm = persist.tile([P, SMW], DT)   # slot-major x0 (beta-scaled)
            a1 = persist.tile([P, SMW], DT)     # slot-major activation buf
            # tail-tile pad lanes (partitions >= TAIL of the last 128-col
            # block) are never written by the tiled producers but are read
            # by full-width elementwise ops — zero them once.
            nc.vector.memset(x0sm[:, FT * P:], 0.0)
            nc.vector.memset(a1[:, FT * P:], 0.0)
            xact = persist.tile([P, SMW], DT)   # sigmoid scratch
            xn16 = persist.tile([P, SMW], DT)   # slot-major layer output

            def dma_slot_major_out(eng, bnc, src):
                # src slot-major [P, SMW] -> bnc row-major [R, P]
                eng.dma_start(
                    bnc[:FT * P, :].rearrange("(t p) h -> p t h", p=P),
                    src[:, :FT * P].rearrange("p (t h) -> p t h", h=P))
                eng.dma_start(bnc[FT * P:R, :], src[:TAIL, FT * P:FT * P + P])

            bounce = [dram.tile([R, P], DT, name=f"bounce{i}")
                      for i in range(NIT)]
            x_full = [dram.tile([cfg.N, P], DT, addr_space="Shared",
                                name=f"x_full{i}") for i in range(NIT)]
            mm_in = dram.tile([1, 2], f32)
            mm_out = dram.tile([1, 2], f32, addr_space="Shared")

            # ================= projection =================
            with tc.tile_pool(name="strips", bufs=1) as strip_pool, \
                 tc.tile_pool(name="pwork", bufs=2) as pwork, \
                 tc.tile_pool(name="pps", bufs=2, space="PSUM") as pps_pool, \
                 tc.tile_pool(name="tps", bufs=4, space="PSUM") as tps_pool:
                strips = []
                for k in range(nk):
                    st = strip_pool.tile([P, R], DT, name=f"strip{k}",
                                         tag=f"strip{k}")
                    eng = nc.sync if k % 2 == 0 else nc.scalar
                    eng.dma_start(st[:], featT[k * P:(k + 1) * P, :])
                    strips.append(st)
                yT = strip_pool.tile([P, R], DT, name="yT", tag="yT")
                rmax = pwork.tile([P, 1], f32, name="rmax", tag="rmax")
                rmin = pwork.tile([P, 1], f32, name="rmin", tag="rmin")
                for ch in range(NCHK):
                    w = cw(ch)
                    sl = slice(ch * cfg.PACK, ch * cfg.PACK + w)
                    ps = pps_pool.tile([P, cfg.PACK], f32, name=f"h{ch}",
                                       tag="hps")
                    for k in range(nk):
                        nc.tensor.matmul(ps[:, :w],
                                         lhsT=wp_sb[:, k * P:(k + 1) * P],
                                         rhs=strips[k][:, sl],
                                         start=(k == 0), stop=(k == nk - 1))
                    nc.vector.tensor_copy(yT[:, sl], ps[:, :w])
                    for c2 in range(4 if w == cfg.PACK else (w + P - 1) // P):
                        w2 = min(P, w - c2 * P)
                        t2 = ch * 4 + c2
                        tp = tps_pool.tile([P, P], f32, name=f"x0t{t2}",
                                           tag="tps")
                        nc.tensor.matmul(
                            tp[:w2, :],
                            lhsT=yT[:, ch * cfg.PACK + c2 * P:
                                    ch * cfg.PACK + c2 * P + w2],
                            rhs=idn[:], start=True, stop=True)
                        nc.scalar.activation(x0sm[:w2, t2 * P:(t2 + 1) * P],
                                             tp[:w2, :], AFT.Copy)
                    qmax = pwork.tile([P, 1], f32, name="qmax", tag="qmax")
                    qmin = pwork.tile([P, 1], f32, name="qmin", tag="qmin")
                    nc.vector.tensor_reduce(qmax[:], ps[:, :w], axis=AX.X,
                                            op=ALU.max)
                    nc.vector.tensor_reduce(qmin[:], ps[:, :w], axis=AX.X,
                                            op=ALU.min)
                    if ch == 0:
                        nc.vector.tensor_copy(rmax[:], qmax[:])
                        nc.vector.tensor_copy(rmin[:], qmin[:])
                    else:
                        nc.vector.tensor_tensor(rmax[:], rmax[:], qmax[:],
                                                op=ALU.max)
                        nc.vector.tensor_tensor(rmin[:], rmin[:], qmin[:],
                                                op=ALU.min)
                nc.vector.tensor_copy(mm_sb[:, 0:1], rmax[:])
                nc.vector.tensor_scalar(mm_sb[:, 1:2], rmin[:], -1.0, None,
                                        ALU.mult)
                nc.gpsimd.partition_all_reduce(mm_ar[:], mm_sb[:],
                                               channels=P, reduce_op=RED.max)
                nc.sync.dma_start(mm_in[:], mm_ar[0:1, :])
                nc.gpsimd.collective_compute(
                    "AllReduce", ALU.max, ins=[mm_in.opt()],
                    outs=[mm_out.opt()], replica_groups=rg)
                nc.sync.dma_start(mm_back[:], mm_out[:])
                bc_ps = pps_pool.tile([P, 2], f32, name="bc_ps", tag="hps")
                nc.tensor.matmul(bc_ps[:], lhsT=ones1[:], rhs=mm_back[:],
                                 start=True, stop=True)
                bcast = pwork.tile([P, 2], f32, name="bcast", tag="qmin")
                nc.vector.tensor_copy(bcast[:], bc_ps[:])
                sden = pwork.tile([P, 1], f32, name="sden", tag="qmax")
                nc.vector.tensor_tensor(sden[:], bcast[:, 0:1], bcast[:, 1:2],
                                        op=ALU.add)
                nc.vector.tensor_scalar(sden[:], sden[:], 1e-8, None, ALU.add)
                nc.vector.reciprocal(sfac[:], sden[:])
                nc.vector.tensor_tensor(bfac[:], bcast[:, 1:2], sfac[:],
                                        op=ALU.mult)
                nc.vector.tensor_scalar(x0sm[:], x0sm[:], sfac[:], bfac[:],
                                        ALU.mult, ALU.add)
                dma_slot_major_out(nc.sync, bounce[0], x0sm)
            nc.gpsimd.collective_compute(
                "AllGather", ALU.bypass, ins=[bounce[0].opt()],
                outs=[x_full[0].opt()], replica_groups=rg)

            # ================= conv layers =================
            with tc.tile_pool(name="gpool", bufs=16) as gpool, \
                 tc.tile_pool(name="ywork", bufs=3) as ywork, \
                 tc.tile_pool(name="yps", bufs=3, space="PSUM") as yps_pool, \
                 tc.tile_pool(name="tps2", bufs=4, space="PSUM") as tps2_pool:
                beta_prev = 1.0
                for l in range(NIT):
                    last = l == NIT - 1
                    beta = min(0.5, (l + 1) / cfg.NLAYERS * 0.5)
                    c1 = float((1.0 - beta) * E1)
                    nc.vector.tensor_scalar(x0sm[:], x0sm[:],
                                            float(beta / beta_prev), None,
                                            ALU.mult)
                    beta_prev = beta
                    xf = x_full[l][:]
                    halves = [xf[:HALF, :], xf[HALF:, :]]
                    gts = []
                    if l == 0:
                        build_program._gq = 0
                    for (h, off, n) in calls:
                        gt = gpool.tile([P, GCAP], DT,
                                        name=f"g{l}_{off}", tag="g")
                        nc.gpsimd.dma_gather(
                            gt[:, :n].rearrange("p (n e) -> p n e", e=P),
                            halves[h],
                            idx_sb[:, off // 16:(off + n) // 16],
                            num_idxs=n, num_idxs_reg=n,
                            elem_size=P, queue_num=build_program._gq % 4)
                        build_program._gq += 1
                        gts.append(gt)
                    for k in range(NPACK):
                        wk = cw(k)
                        n_mm = sum(len(sch.s_cell[(k, h)]) for h in range(2))
                        ps = yps_pool.tile([P, cfg.PACK], f32,
                                           name=f"y{l}_{k}", tag="yps")
                        nc.vector.memset(ps[:, :wk], 0.0)
                        mi = 0
                        for h in range(2):
                            base = sch.cell_off[(k, h)]
                            for ci, (co, wdt, soff) in enumerate(
                                    sch.s_cell[(k, h)]):
                                cidx, j = chunk_call[base + ci * P]
                                lv = gts[cidx][:, j * P:(j + 1) * P]
                                wdt2 = min(wdt, wk - co)
                                nc.tensor.matmul(
                                    ps[:, co:co + wdt2],
                                    lhsT=lv,
                                    rhs=s_sb[:, soff:soff + wdt2],
                                    start=False, stop=(mi == n_mm - 1),
                                    skip_group_check=True)
                                mi += 1
                        yraw = ywork.tile([P, cfg.PACK], DT,
                                          name=f"yr{l}_{k}", tag="yr")
                        nc.vector.tensor_copy(yraw[:, :wk], ps[:, :wk])
                        # transform + transpose: per 128-dest chunk
                        for c in range((wk + P - 1) // P):
                            w = min(P, wk - c * P)
                            gcol = k * cfg.PACK + c * P
                            tp = tps2_pool.tile([P, P], f32,
                                                name=f"t{l}_{k}_{c}",
                                                tag="tps2")
                            nc.tensor.matmul(
                                tp[:w, :], lhsT=yraw[:, c * P:c * P + w],
                                rhs=wh_sb[:, l * P:(l + 1) * P],
                                start=True, stop=True)
                            nc.scalar.activation(
                                a1[:w, gcol:gcol + P], tp[:w, :], AFT.Relu,
                                bias=b_relu[:w], scale=INV08)
                    # activation chain (slot-major, full width, fp16)
                    nc.vector.tensor_scalar(a1[:], a1[:], 1.0, c1,
                                            ALU.min, ALU.mult)
                    nc.scalar.activation(xact[:], a1[:], AFT.Sigmoid,
                                         scale=float(-1.0 / c1))
                    nc.vector.tensor_tensor(a1[:], a1[:], xact[:],
                                            op=ALU.mult)
                    nc.vector.tensor_tensor(xn16[:], a1[:], x0sm[:],
                                            op=ALU.add)
                    if not last:
                        dma_slot_major_out(nc.sync, bounce[l + 1], xn16)
                        nc.gpsimd.collective_compute(
                            "AllGather", ALU.bypass, ins=[bounce[l + 1].opt()],
                            outs=[x_full[l + 1].opt()], replica_groups=rg)

            # ================= output stage =================
            with tc.tile_pool(name="ow", bufs=2) as ow, \
                 tc.tile_pool(name="ops", bufs=2, space="PSUM") as ops_pool:
                for t in range(NT):
                    w = min(P, R - t * P)
                    tp = ops_pool.tile([P, P], f32, name=f"xt{t}", tag="oxt")
                    nc.tensor.matmul(
                        tp[:, :w],
                        lhsT=xn16[:w, t * P:(t + 1) * P],
                        rhs=idn[:w, :w], start=True, stop=True)
                    xnT = ow.tile([P, P], DT, name="xnT", tag="xnT")
                    nc.vector.tensor_copy(xnT[:, :w], tp[:, :w])
                    lg = ops_pool.tile([P, cfg.NCLASS], f32,
                                       name=f"lg{t}", tag="olg")
                    nc.tensor.matmul(lg[:w, :], lhsT=xnT[:, :w],
                                     rhs=wo_sb[:], start=True, stop=True)
                    # logits are bounded (xn in [0,1], small W_out), so
                    # exp/sum is stable without the max subtraction
                    ex = ow.tile([P, cfg.NCLASS], f32, name="ex", tag="ex")
                    sume = ow.tile([P, 1], f32, name="sume", tag="sume")
                    nc.scalar.activation(ex[:w], lg[:w, :], AFT.Exp,
                                         accum_out=sume[:w])
                    lse = ow.tile([P, 1], f32, name="lse", tag="lse")
                    nc.scalar.activation(lse[:w], sume[:w], AFT.Ln)
                    res = ow.tile([P, cfg.NCLASS], f32, name="res", tag="ex")
                    nc.vector.tensor_scalar(res[:w], lg[:w, :], lse[:w],
                                            None, ALU.subtract)
                    nc.sync.dma_start(out[t * P:t * P + w, :], res[:w])
    nc.compile()
    return nc


def kernel(**inputs) -> np.ndarray:
    cfg = Cfg()
    features = np.asarray(inputs["features"], np.float32)
    edge_row = np.asarray(inputs["edge_row"], np.int64)
    edge_col = np.asarray(inputs["edge_col"], np.int64)
    W_in = np.asarray(inputs["W_in"], np.float32)
    Ws = np.asarray(inputs["Ws"], np.float32)
    c = np.asarray(inputs["c"], np.float32)
    W_out = np.asarray(inputs["W_out"], np.float32)

    in_maps, sch, dest_of = preprocess(cfg, features, edge_row, edge_col,
                                       W_in, Ws, c, W_out)
    nc = build_program(cfg, sch)

    import os
    from concourse import bass_utils
    res = bass_utils.run_bass_kernel_spmd(
        nc, in_maps, core_ids=list(range(cfg.n_cores)),
        trace=bool(os.environ.get("GNN_TRACE")))
    kernel.last_result = res
    out = np.empty((cfg.N, cfg.NCLASS), np.float32)
    for d in range(cfg.n_cores):
        out[d * cfg.R + dest_of[d]] = res.results[d]["out"]
    return out
